# revision 19
# baseline (speedup 1.0000x reference)
"""Tensor-parallel GQA multi-head attention for 8 Trainium2 NeuronCores.

Sharding: query heads (16) split 2-per-core; each core needs exactly one
KV head (GQA group); wq/wk/wv column-parallel, wo row-parallel; the
all-reduce after wo is done host-side (sum of 8 partial outputs).

Per-core layout strategy: activations kept transposed (feature dim on
partitions, tokens on the free axis) so every matmul contracts over the
partition dim with N=512 streams:
  QT/KT = W^T-chunks (lhsT) x xT (rhs)         [dh, tokens]
  S^T   = KT-chunk (lhsT) x QT (rhs)           [s, t]  (causal superblocks)
  P^T   = exp(S^T + causal mask)               (no max-subtraction: scores
                                                are bounded ~N(0, 1/9))
  l     = ones x P^T (column sums via PE)      [1, t]
  avT   = V-chunk (lhsT) x P^T (rhs)           [dh, t]; scaled by 1/l
  out   = avT-chunk (lhsT) x woT (rhs)         [t, d] partial, DMA'd out
"""

import numpy as np

B, T, D, H, KV = 2, 2048, 2048, 16, 4
DH = 128
NCORES = 8
HPC = H // NCORES          # 2 query heads per core
BT = B * T                 # 4096
ND = D // 128              # 16 contraction chunks
NSB = T // 512             # 4 causal superblocks per batch
NTC = BT // 128            # 32 output token chunks
ROPE_BASE = 10000.0
NEG = -1.0e4

_cache = {}


def _ensure_path():
    try:
        import concourse.bass  # noqa: F401
    except ImportError:
        import sys
        for p in ("/opt/trn_rl_repo", "/root/.axon_site/_ro/trn_rl_repo"):
            if p not in sys.path:
                sys.path.insert(0, p)
        import concourse.bass  # noqa: F401


def _split_multi_waits(nc, mybir, max_waits=1):
    """This container's walrus rejects >1 sync-wait on one instruction
    (seen on the Tile tail drain). Move extra waits onto preceding NoOps
    on the same engine; per-engine program order preserves semantics."""
    for bb in nc.main_func.blocks:
        new_insts = []
        for ins in bb.instructions:
            si = getattr(ins, "sync_info", None)
            if si is not None and si.on_wait and len(si.on_wait) > max_waits:
                waits = list(si.on_wait)
                extra, keep = waits[:-max_waits], waits[-max_waits:]
                for w in extra:
                    new_insts.append(
                        mybir.InstNoOp(
                            name=nc.get_next_instruction_name(),
                            sync_info=mybir.SyncInfo(on_wait=[w], on_update=[]),
                            bass_nofuse=True,
                            engine=ins.engine,
                            ins=[],
                            outs=[],
                        )
                    )
                si.on_wait = keep
            new_insts.append(ins)
        bb.instructions = new_insts


def _build(split_waits=True, use_f32r=True):
    _ensure_path()
    import concourse.bass as bass
    import concourse.mybir as mybir
    import concourse.tile as tile
    from concourse.masks import make_identity

    f32 = mybir.dt.float32
    fr = mybir.dt.float32r if use_f32r else f32
    nc = bass.Bass()

    xT = nc.declare_dram_parameter("xT", [D, BT], fr, isOutput=False)
    wqT = nc.declare_dram_parameter("wqT", [D, HPC * DH], fr, isOutput=False)
    wkT = nc.declare_dram_parameter("wkT", [D, DH], fr, isOutput=False)
    wvT = nc.declare_dram_parameter("wvT", [D, DH], fr, isOutput=False)
    woT = nc.declare_dram_parameter("woT", [HPC * DH, D], fr, isOutput=False)
    cosT = nc.declare_dram_parameter("cosT", [DH, T], f32, isOutput=False)
    sinT = nc.declare_dram_parameter("sinT", [DH, T], f32, isOutput=False)
    out = nc.declare_dram_parameter("out", [BT, D], f32, isOutput=True)

    with nc.allow_low_precision(reason="float32r fast matmul path"), \
         tile.TileContext(nc) as tc:
        with tc.tile_pool(name="persist", bufs=1) as P:
            ident = P.tile([128, 128], f32, tag="ident")
            maskT = P.tile([128, 128], f32, tag="maskT")
            ones = P.tile([128, 1], fr, tag="ones")
            ones_r = P.tile([1, 128], fr, tag="ones_r")
            ones_f = P.tile([128, 1], f32, tag="ones_f")
            ones_rf = P.tile([1, 128], f32, tag="ones_rf")
            make_identity(nc, ident[:])
            # S^T diag block mask: keep (s_local - t_local) <= 0, else -1e4
            nc.gpsimd.memset(maskT[:], 0.0)
            # keep where (t_local - s_local) >= 0, i.e. s <= t
            nc.gpsimd.affine_select(
                out=maskT[:],
                in_=maskT[:],
                compare_op=mybir.AluOpType.is_ge,
                fill=NEG,
                base=0,
                pattern=[[1, 128]],
                channel_multiplier=-1,
            )
            nc.gpsimd.memset(ones_f[:], 1.0)
            nc.gpsimd.memset(ones_rf[:], 1.0)
            nc.vector.tensor_copy(ones[:], ones_f[:])
            nc.vector.tensor_copy(ones_r[:], ones_rf[:])

            QT = [P.tile([128, BT], fr, tag=f"qt{h}", name=f"qt{h}") for h in range(HPC)]
            KT = P.tile([128, BT], fr, tag="kt")
            VT = P.tile([128, BT], f32, tag="vt")
            Vn = P.tile([128, BT], fr, tag="vn")
            AVT = [P.tile([128, BT], fr, tag=f"avt{h}", name=f"avt{h}") for h in range(HPC)]
            rr = P.tile([1, BT], fr, tag="rr")  # 1/denominator, reused per head

            # ---------- phase A: QKV projections + RoPE ----------
            with tc.tile_pool(name="wpool", bufs=1) as WP, \
                 tc.tile_pool(name="tabs", bufs=1) as TB, \
                 tc.tile_pool(name="xp", bufs=2) as XP, \
                 tc.tile_pool(name="ropesc", bufs=2) as RS, \
                 tc.tile_pool(name="psA", bufs=1, space="PSUM") as PSA:
                wq_sb = WP.tile([128, ND * HPC * DH], fr, tag="wq")
                wk_sb = WP.tile([128, ND * DH], fr, tag="wk")
                wv_sb = WP.tile([128, ND * DH], fr, tag="wv")
                nc.sync.dma_start(
                    out=wq_sb[:].rearrange("p (c m) -> p c m", c=ND),
                    in_=wqT[:, :].rearrange("(c p) m -> p c m", p=128))
                nc.sync.dma_start(
                    out=wk_sb[:].rearrange("p (c m) -> p c m", c=ND),
                    in_=wkT[:, :].rearrange("(c p) m -> p c m", p=128))
                nc.sync.dma_start(
                    out=wv_sb[:].rearrange("p (c m) -> p c m", c=ND),
                    in_=wvT[:, :].rearrange("(c p) m -> p c m", p=128))
                cos_sb = TB.tile([128, T], f32, tag="cos")
                sin_sb = TB.tile([128, T], f32, tag="sin")
                nc.sync.dma_start(out=cos_sb[:], in_=cosT[:, :])
                nc.sync.dma_start(out=sin_sb[:], in_=sinT[:, :])

                def rope_drain(dst, ps, tcol, tabcol):
                    # dst[:, tcol] = ps * cos + rotate_half(ps) * sin_signed
                    qraw = RS.tile([128, 1024], f32, tag="qraw")
                    rot = RS.tile([128, 1024], f32, tag="rot")
                    nc.scalar.copy(qraw[:, :], ps[:, :])
                    nc.sync.dma_start(out=rot[0:64, :], in_=qraw[64:128, :])
                    nc.sync.dma_start(out=rot[64:128, :], in_=qraw[0:64, :])
                    nc.vector.tensor_mul(dst[:, tcol], qraw[:, :], cos_sb[:, tabcol])
                    nc.vector.tensor_mul(rot[:, :], rot[:, :], sin_sb[:, tabcol])
                    nc.vector.tensor_add(dst[:, tcol], dst[:, tcol], rot[:, :])

                for tq in range(BT // 1024):
                    pq = [PSA.tile([128, 1024], f32, tag=f"pq{h}", name=f"pq{h}") for h in range(HPC)]
                    pk = PSA.tile([128, 1024], f32, tag="pk")
                    pv = PSA.tile([128, 1024], f32, tag="pv")
                    for dc in range(ND):
                        xt = XP.tile([128, 1024], fr, tag="x")
                        nc.sync.dma_start(
                            out=xt[:],
                            in_=xT[dc * 128:(dc + 1) * 128, tq * 1024:(tq + 1) * 1024])
                        st, sp = (dc == 0), (dc == ND - 1)
                        for half in range(2):
                            hs = slice(half * 512, (half + 1) * 512)
                            for h in range(HPC):
                                nc.tensor.matmul(
                                    pq[h][:, hs],
                                    lhsT=wq_sb[:, dc * 256 + h * 128: dc * 256 + (h + 1) * 128],
                                    rhs=xt[:, hs], start=st, stop=sp)
                            nc.tensor.matmul(
                                pk[:, hs], lhsT=wk_sb[:, dc * 128:(dc + 1) * 128],
                                rhs=xt[:, hs], start=st, stop=sp)
                            nc.tensor.matmul(
                                pv[:, hs], lhsT=wv_sb[:, dc * 128:(dc + 1) * 128],
                                rhs=xt[:, hs], start=st, stop=sp)
                    tcol = slice(tq * 1024, (tq + 1) * 1024)
                    tb0 = (tq % (T // 1024)) * 1024
                    tabcol = slice(tb0, tb0 + 1024)
                    for h in range(HPC):
                        rope_drain(QT[h], pq[h], tcol, tabcol)
                    rope_drain(KT, pk, tcol, tabcol)
                    nc.scalar.copy(VT[:, tcol], pv[:, :])

            # ---------- phase B0: V^T -> V (natural, s on partitions) ----------
            with tc.tile_pool(name="psB0", bufs=4, space="PSUM") as PSB0:
                for vc in range(BT // 128):
                    vps = PSB0.tile([128, 128], f32, tag="vtp")
                    nc.tensor.transpose(vps[:], VT[:, vc * 128:(vc + 1) * 128], ident[:])
                    nc.scalar.copy(Vn[:, vc * 128:(vc + 1) * 128], vps[:])

            # ---------- phase B: attention ----------
            with tc.tile_pool(name="ptp", bufs=3) as PTP, \
                 tc.tile_pool(name="rbcp", bufs=1, space="PSUM") as RBCP, \
                 tc.tile_pool(name="psSt", bufs=3, space="PSUM") as PSST, \
                 tc.tile_pool(name="psL", bufs=1, space="PSUM") as PSL, \
                 tc.tile_pool(name="psAv", bufs=2, space="PSUM") as PSAV:
                for h in range(HPC):
                    for b in range(B):
                        for tsb in range(NSB):
                            n_sc = (tsb + 1) * 4
                            tg = slice(b * T + tsb * 512, b * T + (tsb + 1) * 512)
                            av_ps = PSAV.tile([128, 512], f32, tag="av")
                            l_ps = PSL.tile([1, 512], f32, tag="l")
                            for sc in range(n_sc):
                                st_ps = PSST.tile([128, 512], f32, tag="st")
                                nc.tensor.matmul(
                                    st_ps[:],
                                    lhsT=KT[:, b * T + sc * 128: b * T + (sc + 1) * 128],
                                    rhs=QT[h][:, tg], start=True, stop=True)
                                sc_rel = sc - tsb * 4
                                if sc_rel >= 0:
                                    for tj in range(4):
                                        blk = st_ps[:, tj * 128:(tj + 1) * 128]
                                        if tj < sc_rel:
                                            nc.vector.memset(blk, NEG)
                                        elif tj == sc_rel:
                                            nc.vector.tensor_add(blk, blk, maskT[:])
                                pt = PTP.tile([128, 512], fr, tag="pt")
                                nc.scalar.activation(
                                    pt[:], st_ps[:], mybir.ActivationFunctionType.Exp)
                                nc.tensor.matmul(
                                    l_ps[:], lhsT=ones[:], rhs=pt[:],
                                    start=(sc == 0), stop=(sc == n_sc - 1))
                                nc.tensor.matmul(
                                    av_ps[:],
                                    lhsT=Vn[:, b * T + sc * 128: b * T + (sc + 1) * 128],
                                    rhs=pt[:], start=(sc == 0), stop=(sc == n_sc - 1))
                            nc.vector.reciprocal(rr[0:1, tg], l_ps[:])
                            nc.scalar.copy(AVT[h][:, tg], av_ps[:])
                    # normalize this head's avT by the softmax denominators
                    for c in range(BT // 512):
                        cols = slice(c * 512, (c + 1) * 512)
                        rbc = RBCP.tile([128, 512], f32, tag="rbc")
                        nc.tensor.matmul(
                            rbc[:], lhsT=ones_r[:], rhs=rr[0:1, cols],
                            start=True, stop=True)
                        nc.vector.tensor_mul(AVT[h][:, cols], AVT[h][:, cols], rbc[:])

            # ---------- phase C: output projection (partial; host sums cores) ----
            with tc.tile_pool(name="wop", bufs=1) as WOP, \
                 tc.tile_pool(name="osbp", bufs=3) as OSBP, \
                 tc.tile_pool(name="psC", bufs=2, space="PSUM") as PSC:
                wo_sb = WOP.tile([128, HPC * D], fr, tag="wo")
                nc.sync.dma_start(
                    out=wo_sb[:].rearrange("p (c n) -> p c n", c=HPC),
                    in_=woT[:, :].rearrange("(c p) n -> p c n", p=128))
                for tcx in range(NTC):
                    wo_ps = PSC.tile([128, D], f32, tag="wops")
                    for h in range(HPC):
                        for ndc in range(4):
                            ns = slice(ndc * 512, (ndc + 1) * 512)
                            nc.tensor.matmul(
                                wo_ps[:, ns],
                                lhsT=AVT[h][:, tcx * 128:(tcx + 1) * 128],
                                rhs=wo_sb[:, h * D + ndc * 512: h * D + (ndc + 1) * 512],
                                start=(h == 0), stop=(h == HPC - 1))
                    osb = OSBP.tile([128, D], f32, tag="osb")
                    nc.vector.tensor_copy(osb[:], wo_ps[:])
                    nc.sync.dma_start(
                        out=out[tcx * 128:(tcx + 1) * 128, :], in_=osb[:])

    if split_waits:
        _split_multi_waits(nc, mybir)
    return nc


def _host_inputs(x, wq, wk, wv, wo):
    xT = np.ascontiguousarray(x.reshape(BT, D).T)
    half = DH // 2
    inv = (1.0 / (ROPE_BASE ** (np.arange(half, dtype=np.float32) / half))).astype(np.float32)
    ang = np.arange(T, dtype=np.float32)[:, None] * inv[None, :]          # (T, 64)
    c = np.cos(ang).T.astype(np.float32)                                  # (64, T)
    s = np.sin(ang).T.astype(np.float32)
    cosT = np.ascontiguousarray(np.concatenate([c, c], axis=0))           # (128, T)
    sinT = np.ascontiguousarray(np.concatenate([-s, s], axis=0))          # signed
    scale = np.float32(1.0 / np.sqrt(DH))
    in_maps = []
    for core in range(NCORES):
        kvh = core // 2
        in_maps.append({
            "xT": xT,
            "wqT": np.ascontiguousarray((wq[core * HPC * DH:(core + 1) * HPC * DH, :] * scale).T),
            "wkT": np.ascontiguousarray(wk[kvh * DH:(kvh + 1) * DH, :].T),
            "wvT": np.ascontiguousarray(wv[kvh * DH:(kvh + 1) * DH, :].T),
            "woT": np.ascontiguousarray(wo[:, core * HPC * DH:(core + 1) * HPC * DH].T),
            "cosT": cosT,
            "sinT": sinT,
        })
    return in_maps


def kernel(x, wq, wk, wv, wo):
    _ensure_path()
    from concourse.bass_utils import run_bass_kernel_spmd

    x = np.asarray(x, dtype=np.float32)
    wq = np.asarray(wq, dtype=np.float32)
    wk = np.asarray(wk, dtype=np.float32)
    wv = np.asarray(wv, dtype=np.float32)
    wo = np.asarray(wo, dtype=np.float32)

    if "nc" not in _cache:
        _cache["nc"] = _build()
    nc = _cache["nc"]

    in_maps = _host_inputs(x, wq, wk, wv, wo)
    res = run_bass_kernel_spmd(nc, in_maps, list(range(NCORES)))
    acc = res.results[0]["out"].astype(np.float32)
    for cidx in range(1, NCORES):
        acc = acc + res.results[cidx]["out"]
    return acc.reshape(B, T, D)


# revision 28
# speedup vs baseline: 1.2337x; 1.2337x over previous
"""Tensor-parallel GQA multi-head attention for 8 Trainium2 NeuronCores.

Sharding: query heads (16) split 2-per-core; each core needs exactly one
KV head (GQA group); wq/wk/wv column-parallel, wo row-parallel; the
all-reduce after wo is done host-side (sum of 8 partial outputs).

Per-core layout strategy: activations kept transposed (feature dim on
partitions, tokens on the free axis) so every matmul contracts over the
partition dim with N=512 streams:
  QT/KT = W^T-chunks (lhsT) x xT (rhs)         [dh, tokens]
  S^T   = KT-chunk (lhsT) x QT (rhs)           [s, t]  (causal superblocks)
  P^T   = exp(S^T + causal mask)               (no max-subtraction: scores
                                                are bounded ~N(0, 1/9))
  l     = ones x P^T (column sums via PE)      [1, t]
  avT   = V-chunk (lhsT) x P^T (rhs)           [dh, t]; scaled by 1/l
  out   = avT-chunk (lhsT) x woT (rhs)         [t, d] partial, DMA'd out
"""

import numpy as np

B, T, D, H, KV = 2, 2048, 2048, 16, 4
DH = 128
NCORES = 8
HPC = H // NCORES          # 2 query heads per core
BT = B * T                 # 4096
ND = D // 128              # 16 contraction chunks
NSB = T // 512             # 4 causal superblocks per batch
NTC = BT // 128            # 32 output token chunks
ROPE_BASE = 10000.0
NEG = -1.0e4

_cache = {}


def _ensure_path():
    try:
        import concourse.bass  # noqa: F401
    except ImportError:
        import sys
        for p in ("/opt/trn_rl_repo", "/root/.axon_site/_ro/trn_rl_repo"):
            if p not in sys.path:
                sys.path.insert(0, p)
        import concourse.bass  # noqa: F401


def _split_multi_waits(nc, mybir, max_waits=1):
    """This container's walrus rejects >1 sync-wait on one instruction
    (seen on the Tile tail drain). Move extra waits onto preceding NoOps
    on the same engine; per-engine program order preserves semantics."""
    for bb in nc.main_func.blocks:
        new_insts = []
        for ins in bb.instructions:
            si = getattr(ins, "sync_info", None)
            if si is not None and si.on_wait and len(si.on_wait) > max_waits:
                waits = list(si.on_wait)
                extra, keep = waits[:-max_waits], waits[-max_waits:]
                for w in extra:
                    new_insts.append(
                        mybir.InstNoOp(
                            name=nc.get_next_instruction_name(),
                            sync_info=mybir.SyncInfo(on_wait=[w], on_update=[]),
                            bass_nofuse=True,
                            engine=ins.engine,
                            ins=[],
                            outs=[],
                        )
                    )
                si.on_wait = keep
            new_insts.append(ins)
        bb.instructions = new_insts


def _build(split_waits=True, use_f32r=True):
    _ensure_path()
    import concourse.bass as bass
    import concourse.mybir as mybir
    import concourse.tile as tile
    from concourse.masks import make_identity

    f32 = mybir.dt.float32
    fr = mybir.dt.float32r if use_f32r else f32
    nc = bass.Bass()

    xT = nc.declare_dram_parameter("xT", [D, BT], fr, isOutput=False)
    wqT = nc.declare_dram_parameter("wqT", [D, HPC * DH], fr, isOutput=False)
    wkT = nc.declare_dram_parameter("wkT", [D, DH], fr, isOutput=False)
    wvT = nc.declare_dram_parameter("wvT", [D, DH], fr, isOutput=False)
    woT = nc.declare_dram_parameter("woT", [HPC * DH, D], fr, isOutput=False)
    cosT = nc.declare_dram_parameter("cosT", [DH, T], f32, isOutput=False)
    rotMT = nc.declare_dram_parameter("rotMT", [DH, DH], fr, isOutput=False)
    sinT = nc.declare_dram_parameter("sinT", [DH, T], f32, isOutput=False)
    out = nc.declare_dram_parameter("out", [BT, D], f32, isOutput=True)

    with nc.allow_low_precision(reason="float32r fast matmul path"), \
         tile.TileContext(nc) as tc:
        with tc.tile_pool(name="persist", bufs=1) as P:
            ident = P.tile([128, 128], f32, tag="ident")
            maskT = P.tile([128, 128], f32, tag="maskT")
            ones = P.tile([128, 1], fr, tag="ones")
            ones_r = P.tile([1, 128], fr, tag="ones_r")
            ones_f = P.tile([128, 1], f32, tag="ones_f")
            ones_rf = P.tile([1, 128], f32, tag="ones_rf")
            make_identity(nc, ident[:])
            # S^T diag block mask: keep (s_local - t_local) <= 0, else -1e4
            nc.gpsimd.memset(maskT[:], 0.0)
            # keep where (t_local - s_local) >= 0, i.e. s <= t
            nc.gpsimd.affine_select(
                out=maskT[:],
                in_=maskT[:],
                compare_op=mybir.AluOpType.is_ge,
                fill=NEG,
                base=0,
                pattern=[[1, 128]],
                channel_multiplier=-1,
            )
            nc.gpsimd.memset(ones_f[:], 1.0)
            nc.gpsimd.memset(ones_rf[:], 1.0)
            nc.vector.tensor_copy(ones[:], ones_f[:])
            nc.vector.tensor_copy(ones_r[:], ones_rf[:])

            rotm_sb = P.tile([128, 128], fr, tag="rotm")
            nc.sync.dma_start(out=rotm_sb[:], in_=rotMT[:, :])
            cos_sb = P.tile([128, T], f32, tag="cos")
            sin_sb = P.tile([128, T], f32, tag="sin")
            nc.sync.dma_start(out=cos_sb[:], in_=cosT[:, :])
            nc.sync.dma_start(out=sin_sb[:], in_=sinT[:, :])
            QT = [P.tile([128, BT], fr, tag=f"qt{h}", name=f"qt{h}") for h in range(HPC)]
            KT = P.tile([128, BT], fr, tag="kt")
            VT = P.tile([128, BT], f32, tag="vt")
            Vn = P.tile([128, BT], fr, tag="vn")
            AVT = [P.tile([128, BT], fr, tag=f"avt{h}", name=f"avt{h}") for h in range(HPC)]

            # ---------- phase A: QKV projections + RoPE ----------
            with tc.tile_pool(name="wpool", bufs=1) as WP, \
                 tc.tile_pool(name="xp", bufs=4) as XP, \
                 tc.tile_pool(name="psA", bufs=1, space="PSUM") as PSA:
                wq_sb = WP.tile([128, ND * HPC * DH], fr, tag="wq")
                wk_sb = WP.tile([128, ND * DH], fr, tag="wk")
                wv_sb = WP.tile([128, ND * DH], fr, tag="wv")
                nc.sync.dma_start(
                    out=wq_sb[:].rearrange("p (c m) -> p c m", c=ND),
                    in_=wqT[:, :].rearrange("(c p) m -> p c m", p=128))
                nc.sync.dma_start(
                    out=wk_sb[:].rearrange("p (c m) -> p c m", c=ND),
                    in_=wkT[:, :].rearrange("(c p) m -> p c m", p=128))
                nc.sync.dma_start(
                    out=wv_sb[:].rearrange("p (c m) -> p c m", c=ND),
                    in_=wvT[:, :].rearrange("(c p) m -> p c m", p=128))

                for tq in range(BT // 1024):
                    pq = [PSA.tile([128, 1024], f32, tag=f"pq{h}", name=f"pq{h}") for h in range(HPC)]
                    pk = PSA.tile([128, 1024], f32, tag="pk")
                    pv = PSA.tile([128, 1024], f32, tag="pv")
                    for dc in range(ND):
                        xt = XP.tile([128, 1024], fr, tag="x")
                        nc.sync.dma_start(
                            out=xt[:],
                            in_=xT[dc * 128:(dc + 1) * 128, tq * 1024:(tq + 1) * 1024])
                        st, sp = (dc == 0), (dc == ND - 1)
                        for half in range(2):
                            hs = slice(half * 512, (half + 1) * 512)
                            for h in range(HPC):
                                nc.tensor.matmul(
                                    pq[h][:, hs],
                                    lhsT=wq_sb[:, dc * 256 + h * 128: dc * 256 + (h + 1) * 128],
                                    rhs=xt[:, hs], start=st, stop=sp)
                            nc.tensor.matmul(
                                pk[:, hs], lhsT=wk_sb[:, dc * 128:(dc + 1) * 128],
                                rhs=xt[:, hs], start=st, stop=sp)
                            nc.tensor.matmul(
                                pv[:, hs], lhsT=wv_sb[:, dc * 128:(dc + 1) * 128],
                                rhs=xt[:, hs], start=st, stop=sp)
                    tcol = slice(tq * 1024, (tq + 1) * 1024)
                    for h in range(HPC):
                        nc.vector.tensor_copy(QT[h][:, tcol], pq[h][:, :])
                    nc.vector.tensor_copy(KT[:, tcol], pk[:, :])
                    nc.vector.tensor_copy(VT[:, tcol], pv[:, :])
            # ---------- phases A2/B0/B/C merged: rope, V-transpose,
            # attention, and interleaved output projection in one scope ------
            with tc.tile_pool(name="wop", bufs=1) as WOP, \
                 tc.tile_pool(name="ropetmp", bufs=4) as RT2, \
                 tc.tile_pool(name="ptp", bufs=3) as PTP, \
                 tc.tile_pool(name="rrp", bufs=2) as RRP, \
                 tc.tile_pool(name="osbp", bufs=3) as OSBP, \
                 tc.tile_pool(name="psScr", bufs=2, space="PSUM") as PSCR, \
                 tc.tile_pool(name="psSt", bufs=2, space="PSUM") as PSST, \
                 tc.tile_pool(name="psL", bufs=1, space="PSUM") as PSL, \
                 tc.tile_pool(name="psAv", bufs=1, space="PSUM") as PSAV, \
                 tc.tile_pool(name="psC", bufs=1, space="PSUM") as PSC:
                wo_sb = WOP.tile([128, HPC * D], fr, tag="wo")
                nc.sync.dma_start(
                    out=wo_sb[:].rearrange("p (c n) -> p c n", c=HPC),
                    in_=woT[:, :].rearrange("(c p) n -> p c n", p=128))
                # RoPE via PE rotation matmul + V transposes, interleaved by
                # token position so early attention groups unblock first
                for c in range(BT // 512):
                    for tgt in [KT, QT[0], QT[1]]:
                        ccol = slice(c * 512, (c + 1) * 512)
                        tab = slice((c * 512) % T, (c * 512) % T + 512)
                        rot_ps = PSCR.tile([128, 512], f32, tag="scr", name="rot_ps")
                        nc.tensor.matmul(rot_ps[:], lhsT=rotm_sb[:],
                                         rhs=tgt[:, ccol], start=True, stop=True)
                        rtmp = RT2.tile([128, 512], f32, tag="rtmp")
                        nc.vector.tensor_mul(rtmp[:], rot_ps[:], sin_sb[:, tab])
                        nc.vector.tensor_mul(tgt[:, ccol], tgt[:, ccol], cos_sb[:, tab])
                        nc.gpsimd.tensor_add(tgt[:, ccol], tgt[:, ccol], rtmp[:])
                    for vc in range(c * 4, (c + 1) * 4):
                        vps = PSCR.tile([128, 128], f32, tag="scr", name="vtp")
                        nc.tensor.transpose(vps[:], VT[:, vc * 128:(vc + 1) * 128],
                                            ident[:])
                        nc.vector.tensor_copy(Vn[:, vc * 128:(vc + 1) * 128], vps[:])
                # attention + output projection
                for b in range(B):
                    for tsb in range(NSB):
                        n_sc = (tsb + 1) * 4
                        tg = slice(b * T + tsb * 512, b * T + (tsb + 1) * 512)
                        for h in range(HPC):
                            av_ps = PSAV.tile([128, 512], f32, tag="av")
                            l_ps = PSL.tile([1, 512], f32, tag="l")
                            for sc in range(n_sc):
                                sc_rel = sc - tsb * 4
                                c0 = max(sc_rel, 0) * 128  # first valid t col
                                nv = slice(c0, 512)
                                tgn = slice(b * T + tsb * 512 + c0,
                                            b * T + (tsb + 1) * 512)
                                st_ps = PSST.tile([128, 512], f32, tag="st")
                                nc.tensor.matmul(
                                    st_ps[:, nv],
                                    lhsT=KT[:, b * T + sc * 128: b * T + (sc + 1) * 128],
                                    rhs=QT[h][:, tgn], start=True, stop=True)
                                if sc_rel >= 0:
                                    blk = st_ps[:, c0:c0 + 128]
                                    nc.vector.tensor_add(blk, blk, maskT[:])
                                pt = PTP.tile([128, 512], fr, tag="pt")
                                nc.scalar.activation(
                                    pt[:, nv], st_ps[:, nv],
                                    mybir.ActivationFunctionType.Exp)
                                nc.tensor.matmul(
                                    l_ps[:, nv], lhsT=ones[:], rhs=pt[:, nv],
                                    start=(sc == 0), stop=(sc == n_sc - 1))
                                nc.tensor.matmul(
                                    av_ps[:, nv],
                                    lhsT=Vn[:, b * T + sc * 128: b * T + (sc + 1) * 128],
                                    rhs=pt[:, nv], start=(sc == 0), stop=(sc == n_sc - 1))
                            rr = RRP.tile([1, 512], fr, tag="rr")
                            nc.vector.reciprocal(rr[:], l_ps[:])
                            nc.vector.tensor_copy(AVT[h][:, tg], av_ps[:])
                            rbc = PSCR.tile([128, 512], f32, tag="scr", name="rbc")
                            nc.tensor.matmul(
                                rbc[:], lhsT=ones_r[:], rhs=rr[:],
                                start=True, stop=True)
                            nc.vector.tensor_mul(AVT[h][:, tg], AVT[h][:, tg], rbc[:])
                        # both heads done for this 512-token group: project out
                        for tj in range(4):
                            tcx = (b * T + tsb * 512) // 128 + tj
                            for dhalf in range(2):
                                wo_ps = PSC.tile([128, D // 2], f32, tag="wops")
                                for h in range(HPC):
                                    for ndc in range(2):
                                        ns = slice(ndc * 512, (ndc + 1) * 512)
                                        nc.tensor.matmul(
                                            wo_ps[:, ns],
                                            lhsT=AVT[h][:, tcx * 128:(tcx + 1) * 128],
                                            rhs=wo_sb[:, h * D + dhalf * 1024 + ndc * 512:
                                                      h * D + dhalf * 1024 + (ndc + 1) * 512],
                                            start=(h == 0), stop=(h == HPC - 1))
                                osb = OSBP.tile([128, D // 2], f32, tag="osb")
                                if (tj + dhalf) % 2 == 0:
                                    nc.vector.tensor_copy(osb[:], wo_ps[:])
                                else:
                                    nc.scalar.copy(osb[:], wo_ps[:])
                                nc.sync.dma_start(
                                    out=out[tcx * 128:(tcx + 1) * 128,
                                            dhalf * 1024:(dhalf + 1) * 1024],
                                    in_=osb[:])

    if split_waits:
        _split_multi_waits(nc, mybir)
    return nc


def _host_inputs(x, wq, wk, wv, wo):
    xT = np.ascontiguousarray(x.reshape(BT, D).T)
    half = DH // 2
    inv = (1.0 / (ROPE_BASE ** (np.arange(half, dtype=np.float32) / half))).astype(np.float32)
    ang = np.arange(T, dtype=np.float32)[:, None] * inv[None, :]          # (T, 64)
    c = np.cos(ang).T.astype(np.float32)                                  # (64, T)
    s = np.sin(ang).T.astype(np.float32)
    cosT = np.ascontiguousarray(np.concatenate([c, c], axis=0))           # (128, T)
    sinT = np.ascontiguousarray(np.concatenate([s, s], axis=0))
    rotMT = np.zeros((DH, DH), dtype=np.float32)
    rotMT[np.arange(64), np.arange(64) + 64] = 1.0    # lhsT: rotM[i+64, i] ... rot = rotM @ q
    rotMT[np.arange(64) + 64, np.arange(64)] = -1.0
    scale = np.float32(1.0 / np.sqrt(DH))
    in_maps = []
    for core in range(NCORES):
        kvh = core // 2
        in_maps.append({
            "xT": xT,
            "wqT": np.ascontiguousarray((wq[core * HPC * DH:(core + 1) * HPC * DH, :] * scale).T),
            "wkT": np.ascontiguousarray(wk[kvh * DH:(kvh + 1) * DH, :].T),
            "wvT": np.ascontiguousarray(wv[kvh * DH:(kvh + 1) * DH, :].T),
            "woT": np.ascontiguousarray(wo[:, core * HPC * DH:(core + 1) * HPC * DH].T),
            "cosT": cosT,
            "sinT": sinT,
            "rotMT": rotMT,
        })
    return in_maps


def kernel(x, wq, wk, wv, wo):
    _ensure_path()
    from concourse.bass_utils import run_bass_kernel_spmd

    x = np.asarray(x, dtype=np.float32)
    wq = np.asarray(wq, dtype=np.float32)
    wk = np.asarray(wk, dtype=np.float32)
    wv = np.asarray(wv, dtype=np.float32)
    wo = np.asarray(wo, dtype=np.float32)

    if "nc" not in _cache:
        _cache["nc"] = _build()
    nc = _cache["nc"]

    in_maps = _host_inputs(x, wq, wk, wv, wo)
    res = run_bass_kernel_spmd(nc, in_maps, list(range(NCORES)))
    acc = res.results[0]["out"].astype(np.float32)
    for cidx in range(1, NCORES):
        acc = acc + res.results[cidx]["out"]
    return acc.reshape(B, T, D)


# revision 30
# speedup vs baseline: 56004.1428x; 45396.2102x over previous
"""Tensor-parallel GQA multi-head attention for 8 Trainium2 NeuronCores.

Sharding: query heads (16) split 2-per-core; each core needs exactly one
KV head (GQA group); wq/wk/wv column-parallel, wo row-parallel; the
all-reduce after wo is done host-side (sum of 8 partial outputs).

Per-core layout strategy: activations kept transposed (feature dim on
partitions, tokens on the free axis) so every matmul contracts over the
partition dim with N=512 streams:
  QT/KT = W^T-chunks (lhsT) x xT (rhs)         [dh, tokens]
  S^T   = KT-chunk (lhsT) x QT (rhs)           [s, t]  (causal superblocks)
  P^T   = exp(S^T + causal mask)               (no max-subtraction: scores
                                                are bounded ~N(0, 1/9))
  l     = ones x P^T (column sums via PE)      [1, t]
  avT   = V-chunk (lhsT) x P^T (rhs)           [dh, t]; scaled by 1/l
  out   = avT-chunk (lhsT) x woT (rhs)         [t, d] partial, DMA'd out
"""

import numpy as np

B, T, D, H, KV = 2, 2048, 2048, 16, 4
DH = 128
NCORES = 8
HPC = H // NCORES          # 2 query heads per core
BT = B * T                 # 4096
ND = D // 128              # 16 contraction chunks
NSB = T // 512             # 4 causal superblocks per batch
NTC = BT // 128            # 32 output token chunks
ROPE_BASE = 10000.0
NEG = -1.0e4

_cache = {}


def _ensure_path():
    try:
        import concourse.bass  # noqa: F401
    except ImportError:
        import sys
        for p in ("/opt/trn_rl_repo", "/root/.axon_site/_ro/trn_rl_repo"):
            if p not in sys.path:
                sys.path.insert(0, p)
        import concourse.bass  # noqa: F401


def _split_multi_waits(nc, mybir, max_waits=1):
    """This container's walrus rejects >1 sync-wait on one instruction
    (seen on the Tile tail drain). Move extra waits onto preceding NoOps
    on the same engine; per-engine program order preserves semantics."""
    for bb in nc.main_func.blocks:
        new_insts = []
        for ins in bb.instructions:
            si = getattr(ins, "sync_info", None)
            if si is not None and si.on_wait and len(si.on_wait) > max_waits:
                waits = list(si.on_wait)
                extra, keep = waits[:-max_waits], waits[-max_waits:]
                for w in extra:
                    new_insts.append(
                        mybir.InstNoOp(
                            name=nc.get_next_instruction_name(),
                            sync_info=mybir.SyncInfo(on_wait=[w], on_update=[]),
                            bass_nofuse=True,
                            engine=ins.engine,
                            ins=[],
                            outs=[],
                        )
                    )
                si.on_wait = keep
            new_insts.append(ins)
        bb.instructions = new_insts


def _build(split_waits=True, use_f32r=True):
    _ensure_path()
    import concourse.bass as bass
    import concourse.mybir as mybir
    import concourse.tile as tile
    from concourse.masks import make_identity

    f32 = mybir.dt.float32
    fr = mybir.dt.float32r if use_f32r else f32
    nc = bass.Bass()

    xT = nc.declare_dram_parameter("xT", [D, BT], fr, isOutput=False)
    wqT = nc.declare_dram_parameter("wqT", [D, HPC * DH], fr, isOutput=False)
    wkT = nc.declare_dram_parameter("wkT", [D, DH], fr, isOutput=False)
    wvT = nc.declare_dram_parameter("wvT", [D, DH], fr, isOutput=False)
    woT = nc.declare_dram_parameter("woT", [HPC * DH, D], fr, isOutput=False)
    cosT = nc.declare_dram_parameter("cosT", [DH, T], f32, isOutput=False)
    rotMT = nc.declare_dram_parameter("rotMT", [DH, DH], fr, isOutput=False)
    sinT = nc.declare_dram_parameter("sinT", [DH, T], f32, isOutput=False)
    out = nc.declare_dram_parameter("out", [BT, D], f32, isOutput=True)

    with nc.allow_low_precision(reason="float32r fast matmul path"), \
         tile.TileContext(nc) as tc:
        with tc.tile_pool(name="persist", bufs=1) as P:
            ident = P.tile([128, 128], f32, tag="ident")
            maskT = P.tile([128, 128], f32, tag="maskT")
            ones = P.tile([128, 1], fr, tag="ones")
            ones_r = P.tile([1, 128], fr, tag="ones_r")
            ones_f = P.tile([128, 1], f32, tag="ones_f")
            ones_rf = P.tile([1, 128], f32, tag="ones_rf")
            make_identity(nc, ident[:])
            # S^T diag block mask: keep (s_local - t_local) <= 0, else -1e4
            nc.gpsimd.memset(maskT[:], 0.0)
            # keep where (t_local - s_local) >= 0, i.e. s <= t
            nc.gpsimd.affine_select(
                out=maskT[:],
                in_=maskT[:],
                compare_op=mybir.AluOpType.is_ge,
                fill=NEG,
                base=0,
                pattern=[[1, 128]],
                channel_multiplier=-1,
            )
            nc.gpsimd.memset(ones_f[:], 1.0)
            nc.gpsimd.memset(ones_rf[:], 1.0)
            nc.vector.tensor_copy(ones[:], ones_f[:])
            nc.vector.tensor_copy(ones_r[:], ones_rf[:])

            rotm_sb = P.tile([128, 128], fr, tag="rotm")
            nc.sync.dma_start(out=rotm_sb[:], in_=rotMT[:, :])
            cos_sb = P.tile([128, T], f32, tag="cos")
            sin_sb = P.tile([128, T], f32, tag="sin")
            nc.sync.dma_start(out=cos_sb[:], in_=cosT[:, :])
            nc.sync.dma_start(out=sin_sb[:], in_=sinT[:, :])
            QT = [P.tile([128, BT], fr, tag=f"qt{h}", name=f"qt{h}") for h in range(HPC)]
            KT = P.tile([128, BT], fr, tag="kt")
            VT = P.tile([128, BT], f32, tag="vt")
            Vn = P.tile([128, BT], fr, tag="vn")
            AVT = [P.tile([128, BT], fr, tag=f"avt{h}", name=f"avt{h}") for h in range(HPC)]

            # ---------- phase A: QKV projections + RoPE ----------
            with tc.tile_pool(name="wpool", bufs=1) as WP, \
                 tc.tile_pool(name="xp", bufs=6) as XP, \
                 tc.tile_pool(name="psA", bufs=1, space="PSUM") as PSA:
                wq_sb = WP.tile([128, ND * HPC * DH], fr, tag="wq")
                wk_sb = WP.tile([128, ND * DH], fr, tag="wk")
                wv_sb = WP.tile([128, ND * DH], fr, tag="wv")
                nc.sync.dma_start(
                    out=wq_sb[:].rearrange("p (c m) -> p c m", c=ND),
                    in_=wqT[:, :].rearrange("(c p) m -> p c m", p=128))
                nc.sync.dma_start(
                    out=wk_sb[:].rearrange("p (c m) -> p c m", c=ND),
                    in_=wkT[:, :].rearrange("(c p) m -> p c m", p=128))
                nc.sync.dma_start(
                    out=wv_sb[:].rearrange("p (c m) -> p c m", c=ND),
                    in_=wvT[:, :].rearrange("(c p) m -> p c m", p=128))

                for tq in range(BT // 1024):
                    pq = [PSA.tile([128, 1024], f32, tag=f"pq{h}", name=f"pq{h}") for h in range(HPC)]
                    pk = PSA.tile([128, 1024], f32, tag="pk")
                    pv = PSA.tile([128, 1024], f32, tag="pv")
                    for dc in range(ND):
                        xt = XP.tile([128, 1024], fr, tag="x")
                        nc.sync.dma_start(
                            out=xt[:],
                            in_=xT[dc * 128:(dc + 1) * 128, tq * 1024:(tq + 1) * 1024])
                        st, sp = (dc == 0), (dc == ND - 1)
                        for half in range(2):
                            hs = slice(half * 512, (half + 1) * 512)
                            for h in range(HPC):
                                nc.tensor.matmul(
                                    pq[h][:, hs],
                                    lhsT=wq_sb[:, dc * 256 + h * 128: dc * 256 + (h + 1) * 128],
                                    rhs=xt[:, hs], start=st, stop=sp)
                            nc.tensor.matmul(
                                pk[:, hs], lhsT=wk_sb[:, dc * 128:(dc + 1) * 128],
                                rhs=xt[:, hs], start=st, stop=sp)
                            nc.tensor.matmul(
                                pv[:, hs], lhsT=wv_sb[:, dc * 128:(dc + 1) * 128],
                                rhs=xt[:, hs], start=st, stop=sp)
                    tcol = slice(tq * 1024, (tq + 1) * 1024)
                    for h in range(HPC):
                        nc.vector.tensor_copy(QT[h][:, tcol], pq[h][:, :])
                    nc.vector.tensor_copy(KT[:, tcol], pk[:, :])
                    nc.vector.tensor_copy(VT[:, tcol], pv[:, :])
            # ---------- phases A2/B0/B/C merged: rope, V-transpose,
            # attention, and interleaved output projection in one scope ------
            with tc.tile_pool(name="wop", bufs=1) as WOP, \
                 tc.tile_pool(name="ropetmp", bufs=4) as RT2, \
                 tc.tile_pool(name="ptp", bufs=4) as PTP, \
                 tc.tile_pool(name="rrp", bufs=2) as RRP, \
                 tc.tile_pool(name="osbp", bufs=3) as OSBP, \
                 tc.tile_pool(name="psScr", bufs=2, space="PSUM") as PSCR, \
                 tc.tile_pool(name="psSt", bufs=2, space="PSUM") as PSST, \
                 tc.tile_pool(name="psL", bufs=1, space="PSUM") as PSL, \
                 tc.tile_pool(name="psAv", bufs=1, space="PSUM") as PSAV, \
                 tc.tile_pool(name="psC", bufs=1, space="PSUM") as PSC:
                wo_sb = WOP.tile([128, HPC * D], fr, tag="wo")
                nc.sync.dma_start(
                    out=wo_sb[:].rearrange("p (c n) -> p c n", c=HPC),
                    in_=woT[:, :].rearrange("(c p) n -> p c n", p=128))
                # RoPE via PE rotation matmul + V transposes, interleaved by
                # token position so early attention groups unblock first
                for c in range(BT // 512):
                    for tgt in [KT, QT[0], QT[1]]:
                        ccol = slice(c * 512, (c + 1) * 512)
                        tab = slice((c * 512) % T, (c * 512) % T + 512)
                        rot_ps = PSCR.tile([128, 512], f32, tag="scr", name="rot_ps")
                        nc.tensor.matmul(rot_ps[:], lhsT=rotm_sb[:],
                                         rhs=tgt[:, ccol], start=True, stop=True)
                        rtmp = RT2.tile([128, 512], f32, tag="rtmp")
                        nc.vector.tensor_mul(rtmp[:], rot_ps[:], sin_sb[:, tab])
                        nc.vector.tensor_mul(tgt[:, ccol], tgt[:, ccol], cos_sb[:, tab])
                        nc.gpsimd.tensor_add(tgt[:, ccol], tgt[:, ccol], rtmp[:])
                    for vc in range(c * 4, (c + 1) * 4):
                        vps = PSCR.tile([128, 128], f32, tag="scr", name="vtp")
                        nc.tensor.transpose(vps[:], VT[:, vc * 128:(vc + 1) * 128],
                                            ident[:])
                        nc.vector.tensor_copy(Vn[:, vc * 128:(vc + 1) * 128], vps[:])
                # attention + output projection
                for b in range(B):
                    for tsb in range(NSB):
                        n_sc = (tsb + 1) * 4
                        tg = slice(b * T + tsb * 512, b * T + (tsb + 1) * 512)
                        for h in range(HPC):
                            av_ps = PSAV.tile([128, 512], f32, tag="av")
                            l_ps = PSL.tile([1, 512], f32, tag="l")
                            for sc in range(n_sc):
                                sc_rel = sc - tsb * 4
                                c0 = max(sc_rel, 0) * 128  # first valid t col
                                nv = slice(c0, 512)
                                tgn = slice(b * T + tsb * 512 + c0,
                                            b * T + (tsb + 1) * 512)
                                st_ps = PSST.tile([128, 512], f32, tag="st")
                                nc.tensor.matmul(
                                    st_ps[:, nv],
                                    lhsT=KT[:, b * T + sc * 128: b * T + (sc + 1) * 128],
                                    rhs=QT[h][:, tgn], start=True, stop=True)
                                if sc_rel >= 0:
                                    blk = st_ps[:, c0:c0 + 128]
                                    nc.vector.tensor_add(blk, blk, maskT[:])
                                pt = PTP.tile([128, 512], fr, tag="pt")
                                nc.scalar.activation(
                                    pt[:, nv], st_ps[:, nv],
                                    mybir.ActivationFunctionType.Exp)
                                nc.tensor.matmul(
                                    l_ps[:, nv], lhsT=ones[:], rhs=pt[:, nv],
                                    start=(sc == 0), stop=(sc == n_sc - 1))
                                nc.tensor.matmul(
                                    av_ps[:, nv],
                                    lhsT=Vn[:, b * T + sc * 128: b * T + (sc + 1) * 128],
                                    rhs=pt[:, nv], start=(sc == 0), stop=(sc == n_sc - 1))
                            rr = RRP.tile([1, 512], fr, tag="rr")
                            nc.vector.reciprocal(rr[:], l_ps[:])
                            nc.vector.tensor_copy(AVT[h][:, tg], av_ps[:])
                            rbc = PSCR.tile([128, 512], f32, tag="scr", name="rbc")
                            nc.tensor.matmul(
                                rbc[:], lhsT=ones_r[:], rhs=rr[:],
                                start=True, stop=True)
                            nc.vector.tensor_mul(AVT[h][:, tg], AVT[h][:, tg], rbc[:])
                        # both heads done for this 512-token group: project out
                        for tj in range(4):
                            tcx = (b * T + tsb * 512) // 128 + tj
                            for dhalf in range(2):
                                wo_ps = PSC.tile([128, D // 2], f32, tag="wops")
                                for h in range(HPC):
                                    for ndc in range(2):
                                        ns = slice(ndc * 512, (ndc + 1) * 512)
                                        nc.tensor.matmul(
                                            wo_ps[:, ns],
                                            lhsT=AVT[h][:, tcx * 128:(tcx + 1) * 128],
                                            rhs=wo_sb[:, h * D + dhalf * 1024 + ndc * 512:
                                                      h * D + dhalf * 1024 + (ndc + 1) * 512],
                                            start=(h == 0), stop=(h == HPC - 1))
                                osb = OSBP.tile([128, D // 2], f32, tag="osb")
                                if (tj + dhalf) % 2 == 0:
                                    nc.vector.tensor_copy(osb[:], wo_ps[:])
                                else:
                                    nc.scalar.copy(osb[:], wo_ps[:])
                                nc.sync.dma_start(
                                    out=out[tcx * 128:(tcx + 1) * 128,
                                            dhalf * 1024:(dhalf + 1) * 1024],
                                    in_=osb[:])

    if split_waits:
        _split_multi_waits(nc, mybir)
    return nc


def _host_inputs(x, wq, wk, wv, wo):
    xT = np.ascontiguousarray(x.reshape(BT, D).T)
    half = DH // 2
    inv = (1.0 / (ROPE_BASE ** (np.arange(half, dtype=np.float32) / half))).astype(np.float32)
    ang = np.arange(T, dtype=np.float32)[:, None] * inv[None, :]          # (T, 64)
    c = np.cos(ang).T.astype(np.float32)                                  # (64, T)
    s = np.sin(ang).T.astype(np.float32)
    cosT = np.ascontiguousarray(np.concatenate([c, c], axis=0))           # (128, T)
    sinT = np.ascontiguousarray(np.concatenate([s, s], axis=0))
    rotMT = np.zeros((DH, DH), dtype=np.float32)
    rotMT[np.arange(64), np.arange(64) + 64] = 1.0    # lhsT: rotM[i+64, i] ... rot = rotM @ q
    rotMT[np.arange(64) + 64, np.arange(64)] = -1.0
    scale = np.float32(1.0 / np.sqrt(DH))
    in_maps = []
    for core in range(NCORES):
        kvh = core // 2
        in_maps.append({
            "xT": xT,
            "wqT": np.ascontiguousarray((wq[core * HPC * DH:(core + 1) * HPC * DH, :] * scale).T),
            "wkT": np.ascontiguousarray(wk[kvh * DH:(kvh + 1) * DH, :].T),
            "wvT": np.ascontiguousarray(wv[kvh * DH:(kvh + 1) * DH, :].T),
            "woT": np.ascontiguousarray(wo[:, core * HPC * DH:(core + 1) * HPC * DH].T),
            "cosT": cosT,
            "sinT": sinT,
            "rotMT": rotMT,
        })
    return in_maps


def kernel(x, wq, wk, wv, wo):
    _ensure_path()
    from concourse.bass_utils import run_bass_kernel_spmd

    x = np.asarray(x, dtype=np.float32)
    wq = np.asarray(wq, dtype=np.float32)
    wk = np.asarray(wk, dtype=np.float32)
    wv = np.asarray(wv, dtype=np.float32)
    wo = np.asarray(wo, dtype=np.float32)

    if "nc" not in _cache:
        _cache["nc"] = _build()
    nc = _cache["nc"]

    in_maps = _host_inputs(x, wq, wk, wv, wo)
    res = run_bass_kernel_spmd(nc, in_maps, list(range(NCORES)))
    acc = res.results[0]["out"].astype(np.float32)
    for cidx in range(1, NCORES):
        acc = acc + res.results[cidx]["out"]
    return acc.reshape(B, T, D)


# revision 32
# speedup vs baseline: 56722.9442x; 1.0128x over previous
"""Tensor-parallel GQA multi-head attention for 8 Trainium2 NeuronCores.

Sharding: query heads (16) split 2-per-core; each core needs exactly one
KV head (GQA group); wq/wk/wv column-parallel, wo row-parallel; the
all-reduce after wo is done host-side (sum of 8 partial outputs).

Per-core layout strategy: activations kept transposed (feature dim on
partitions, tokens on the free axis) so every matmul contracts over the
partition dim with N=512 streams:
  QT/KT = W^T-chunks (lhsT) x xT (rhs)         [dh, tokens]
  S^T   = KT-chunk (lhsT) x QT (rhs)           [s, t]  (causal superblocks)
  P^T   = exp(S^T + causal mask)               (no max-subtraction: scores
                                                are bounded ~N(0, 1/9))
  l     = ones x P^T (column sums via PE)      [1, t]
  avT   = V-chunk (lhsT) x P^T (rhs)           [dh, t]; scaled by 1/l
  out   = avT-chunk (lhsT) x woT (rhs)         [t, d] partial, DMA'd out
"""

import numpy as np

B, T, D, H, KV = 2, 2048, 2048, 16, 4
DH = 128
NCORES = 8
HPC = H // NCORES          # 2 query heads per core
BT = B * T                 # 4096
ND = D // 128              # 16 contraction chunks
NSB = T // 512             # 4 causal superblocks per batch
NTC = BT // 128            # 32 output token chunks
ROPE_BASE = 10000.0
NEG = -1.0e4

_cache = {}


def _ensure_path():
    try:
        import concourse.bass  # noqa: F401
    except ImportError:
        import sys
        for p in ("/opt/trn_rl_repo", "/root/.axon_site/_ro/trn_rl_repo"):
            if p not in sys.path:
                sys.path.insert(0, p)
        import concourse.bass  # noqa: F401


def _split_multi_waits(nc, mybir, max_waits=1):
    """This container's walrus rejects >1 sync-wait on one instruction
    (seen on the Tile tail drain). Move extra waits onto preceding NoOps
    on the same engine; per-engine program order preserves semantics."""
    for bb in nc.main_func.blocks:
        new_insts = []
        for ins in bb.instructions:
            si = getattr(ins, "sync_info", None)
            if si is not None and si.on_wait and len(si.on_wait) > max_waits:
                waits = list(si.on_wait)
                extra, keep = waits[:-max_waits], waits[-max_waits:]
                for w in extra:
                    new_insts.append(
                        mybir.InstNoOp(
                            name=nc.get_next_instruction_name(),
                            sync_info=mybir.SyncInfo(on_wait=[w], on_update=[]),
                            bass_nofuse=True,
                            engine=ins.engine,
                            ins=[],
                            outs=[],
                        )
                    )
                si.on_wait = keep
            new_insts.append(ins)
        bb.instructions = new_insts


def _build(split_waits=True, use_f32r=True):
    _ensure_path()
    import concourse.bass as bass
    import concourse.mybir as mybir
    import concourse.tile as tile
    from concourse.masks import make_identity

    f32 = mybir.dt.float32
    fr = mybir.dt.float32r if use_f32r else f32
    nc = bass.Bass()

    xT = nc.declare_dram_parameter("xT", [D, BT], fr, isOutput=False)
    wqT = nc.declare_dram_parameter("wqT", [D, HPC * DH], fr, isOutput=False)
    wkT = nc.declare_dram_parameter("wkT", [D, DH], fr, isOutput=False)
    wvT = nc.declare_dram_parameter("wvT", [D, DH], fr, isOutput=False)
    woT = nc.declare_dram_parameter("woT", [HPC * DH, D], fr, isOutput=False)
    cosT = nc.declare_dram_parameter("cosT", [DH, T], f32, isOutput=False)
    rotMT = nc.declare_dram_parameter("rotMT", [DH, DH], fr, isOutput=False)
    sinT = nc.declare_dram_parameter("sinT", [DH, T], f32, isOutput=False)
    out = nc.declare_dram_parameter("out", [BT, D], f32, isOutput=True)

    with nc.allow_low_precision(reason="float32r fast matmul path"), \
         tile.TileContext(nc) as tc:
        with tc.tile_pool(name="persist", bufs=1) as P:
            ident = P.tile([128, 128], f32, tag="ident")
            maskT = P.tile([128, 128], f32, tag="maskT")
            ones = P.tile([128, 1], fr, tag="ones")
            ones_r = P.tile([1, 128], fr, tag="ones_r")
            ones_f = P.tile([128, 1], f32, tag="ones_f")
            ones_rf = P.tile([1, 128], f32, tag="ones_rf")
            make_identity(nc, ident[:])
            # S^T diag block mask: keep (s_local - t_local) <= 0, else -1e4
            nc.gpsimd.memset(maskT[:], 0.0)
            # keep where (t_local - s_local) >= 0, i.e. s <= t
            nc.gpsimd.affine_select(
                out=maskT[:],
                in_=maskT[:],
                compare_op=mybir.AluOpType.is_ge,
                fill=NEG,
                base=0,
                pattern=[[1, 128]],
                channel_multiplier=-1,
            )
            nc.gpsimd.memset(ones_f[:], 1.0)
            nc.gpsimd.memset(ones_rf[:], 1.0)
            nc.vector.tensor_copy(ones[:], ones_f[:])
            nc.vector.tensor_copy(ones_r[:], ones_rf[:])

            rotm_sb = P.tile([128, 128], fr, tag="rotm")
            cos_sb = P.tile([128, T], f32, tag="cos")
            sin_sb = P.tile([128, T], f32, tag="sin")
            QT = [P.tile([128, BT], fr, tag=f"qt{h}", name=f"qt{h}") for h in range(HPC)]
            KT = P.tile([128, BT], fr, tag="kt")
            VT = P.tile([128, BT], f32, tag="vt")
            Vn = P.tile([128, BT], fr, tag="vn")
            AVT = [P.tile([128, BT], fr, tag=f"avt{h}", name=f"avt{h}") for h in range(HPC)]

            # ---------- phase A: QKV projections + RoPE ----------
            with tc.tile_pool(name="wpool", bufs=1) as WP, \
                 tc.tile_pool(name="xp", bufs=6) as XP, \
                 tc.tile_pool(name="psA", bufs=1, space="PSUM") as PSA:
                wq_sb = WP.tile([128, ND * HPC * DH], fr, tag="wq")
                wk_sb = WP.tile([128, ND * DH], fr, tag="wk")
                wv_sb = WP.tile([128, ND * DH], fr, tag="wv")
                # split weight loads so the first d-chunks land quickly
                for lo, hi in ((0, ND // 4), (ND // 4, ND)):
                    nc.sync.dma_start(
                        out=wq_sb[:, lo * 256: hi * 256].rearrange(
                            "p (c m) -> p c m", c=hi - lo),
                        in_=wqT[lo * 128: hi * 128, :].rearrange(
                            "(c p) m -> p c m", p=128))
                    nc.sync.dma_start(
                        out=wk_sb[:, lo * 128: hi * 128].rearrange(
                            "p (c m) -> p c m", c=hi - lo),
                        in_=wkT[lo * 128: hi * 128, :].rearrange(
                            "(c p) m -> p c m", p=128))
                    nc.sync.dma_start(
                        out=wv_sb[:, lo * 128: hi * 128].rearrange(
                            "p (c m) -> p c m", c=hi - lo),
                        in_=wvT[lo * 128: hi * 128, :].rearrange(
                            "(c p) m -> p c m", p=128))

                for tq in range(BT // 1024):
                    if tq == 1:
                        # defer table loads past the first token-quarter so the
                        # first projection matmuls aren't queued behind them
                        nc.sync.dma_start(out=rotm_sb[:], in_=rotMT[:, :])
                        nc.sync.dma_start(out=cos_sb[:], in_=cosT[:, :])
                        nc.sync.dma_start(out=sin_sb[:], in_=sinT[:, :])
                    pq = [PSA.tile([128, 1024], f32, tag=f"pq{h}", name=f"pq{h}") for h in range(HPC)]
                    pk = PSA.tile([128, 1024], f32, tag="pk")
                    pv = PSA.tile([128, 1024], f32, tag="pv")
                    for dc in range(ND):
                        xt = XP.tile([128, 1024], fr, tag="x")
                        nc.sync.dma_start(
                            out=xt[:],
                            in_=xT[dc * 128:(dc + 1) * 128, tq * 1024:(tq + 1) * 1024])
                        st, sp = (dc == 0), (dc == ND - 1)
                        for half in range(2):
                            hs = slice(half * 512, (half + 1) * 512)
                            for h in range(HPC):
                                nc.tensor.matmul(
                                    pq[h][:, hs],
                                    lhsT=wq_sb[:, dc * 256 + h * 128: dc * 256 + (h + 1) * 128],
                                    rhs=xt[:, hs], start=st, stop=sp)
                            nc.tensor.matmul(
                                pk[:, hs], lhsT=wk_sb[:, dc * 128:(dc + 1) * 128],
                                rhs=xt[:, hs], start=st, stop=sp)
                            nc.tensor.matmul(
                                pv[:, hs], lhsT=wv_sb[:, dc * 128:(dc + 1) * 128],
                                rhs=xt[:, hs], start=st, stop=sp)
                    tcol = slice(tq * 1024, (tq + 1) * 1024)
                    for h in range(HPC):
                        nc.vector.tensor_copy(QT[h][:, tcol], pq[h][:, :])
                    nc.vector.tensor_copy(KT[:, tcol], pk[:, :])
                    nc.vector.tensor_copy(VT[:, tcol], pv[:, :])
            # ---------- phases A2/B0/B/C merged: rope, V-transpose,
            # attention, and interleaved output projection in one scope ------
            with tc.tile_pool(name="wop", bufs=1) as WOP, \
                 tc.tile_pool(name="ropetmp", bufs=4) as RT2, \
                 tc.tile_pool(name="ptp", bufs=4) as PTP, \
                 tc.tile_pool(name="rrp", bufs=2) as RRP, \
                 tc.tile_pool(name="osbp", bufs=3) as OSBP, \
                 tc.tile_pool(name="psScr", bufs=2, space="PSUM") as PSCR, \
                 tc.tile_pool(name="psSt", bufs=2, space="PSUM") as PSST, \
                 tc.tile_pool(name="psL", bufs=1, space="PSUM") as PSL, \
                 tc.tile_pool(name="psAv", bufs=1, space="PSUM") as PSAV, \
                 tc.tile_pool(name="psC", bufs=1, space="PSUM") as PSC:
                wo_sb = WOP.tile([128, HPC * D], fr, tag="wo")
                nc.sync.dma_start(
                    out=wo_sb[:].rearrange("p (c n) -> p c n", c=HPC),
                    in_=woT[:, :].rearrange("(c p) n -> p c n", p=128))
                # RoPE via PE rotation matmul + V transposes, interleaved by
                # token position so early attention groups unblock first
                for c in range(BT // 512):
                    for tgt in [KT, QT[0], QT[1]]:
                        ccol = slice(c * 512, (c + 1) * 512)
                        tab = slice((c * 512) % T, (c * 512) % T + 512)
                        rot_ps = PSCR.tile([128, 512], f32, tag="scr", name="rot_ps")
                        nc.tensor.matmul(rot_ps[:], lhsT=rotm_sb[:],
                                         rhs=tgt[:, ccol], start=True, stop=True)
                        rtmp = RT2.tile([128, 512], f32, tag="rtmp")
                        nc.vector.tensor_mul(rtmp[:], rot_ps[:], sin_sb[:, tab])
                        nc.vector.tensor_mul(tgt[:, ccol], tgt[:, ccol], cos_sb[:, tab])
                        nc.gpsimd.tensor_add(tgt[:, ccol], tgt[:, ccol], rtmp[:])
                    for vc in range(c * 4, (c + 1) * 4):
                        vps = PSCR.tile([128, 128], f32, tag="scr", name="vtp")
                        nc.tensor.transpose(vps[:], VT[:, vc * 128:(vc + 1) * 128],
                                            ident[:])
                        nc.vector.tensor_copy(Vn[:, vc * 128:(vc + 1) * 128], vps[:])
                # attention + output projection
                for b in range(B):
                    for tsb in range(NSB):
                        n_sc = (tsb + 1) * 4
                        tg = slice(b * T + tsb * 512, b * T + (tsb + 1) * 512)
                        for h in range(HPC):
                            av_ps = PSAV.tile([128, 512], f32, tag="av")
                            l_ps = PSL.tile([1, 512], f32, tag="l")
                            for sc in range(n_sc):
                                sc_rel = sc - tsb * 4
                                c0 = max(sc_rel, 0) * 128  # first valid t col
                                nv = slice(c0, 512)
                                tgn = slice(b * T + tsb * 512 + c0,
                                            b * T + (tsb + 1) * 512)
                                st_ps = PSST.tile([128, 512], f32, tag="st")
                                nc.tensor.matmul(
                                    st_ps[:, nv],
                                    lhsT=KT[:, b * T + sc * 128: b * T + (sc + 1) * 128],
                                    rhs=QT[h][:, tgn], start=True, stop=True)
                                if sc_rel >= 0:
                                    blk = st_ps[:, c0:c0 + 128]
                                    nc.vector.tensor_add(blk, blk, maskT[:])
                                pt = PTP.tile([128, 512], fr, tag="pt")
                                nc.scalar.activation(
                                    pt[:, nv], st_ps[:, nv],
                                    mybir.ActivationFunctionType.Exp)
                                nc.tensor.matmul(
                                    l_ps[:, nv], lhsT=ones[:], rhs=pt[:, nv],
                                    start=(sc == 0), stop=(sc == n_sc - 1))
                                nc.tensor.matmul(
                                    av_ps[:, nv],
                                    lhsT=Vn[:, b * T + sc * 128: b * T + (sc + 1) * 128],
                                    rhs=pt[:, nv], start=(sc == 0), stop=(sc == n_sc - 1))
                            rr = RRP.tile([1, 512], fr, tag="rr")
                            nc.vector.reciprocal(rr[:], l_ps[:])
                            nc.vector.tensor_copy(AVT[h][:, tg], av_ps[:])
                            rbc = PSCR.tile([128, 512], f32, tag="scr", name="rbc")
                            nc.tensor.matmul(
                                rbc[:], lhsT=ones_r[:], rhs=rr[:],
                                start=True, stop=True)
                            nc.vector.tensor_mul(AVT[h][:, tg], AVT[h][:, tg], rbc[:])
                        # both heads done for this 512-token group: project out
                        for tj in range(4):
                            tcx = (b * T + tsb * 512) // 128 + tj
                            for dhalf in range(2):
                                wo_ps = PSC.tile([128, D // 2], f32, tag="wops")
                                for h in range(HPC):
                                    for ndc in range(2):
                                        ns = slice(ndc * 512, (ndc + 1) * 512)
                                        nc.tensor.matmul(
                                            wo_ps[:, ns],
                                            lhsT=AVT[h][:, tcx * 128:(tcx + 1) * 128],
                                            rhs=wo_sb[:, h * D + dhalf * 1024 + ndc * 512:
                                                      h * D + dhalf * 1024 + (ndc + 1) * 512],
                                            start=(h == 0), stop=(h == HPC - 1))
                                osb = OSBP.tile([128, D // 2], f32, tag="osb")
                                if (tj + dhalf) % 2 == 0:
                                    nc.vector.tensor_copy(osb[:], wo_ps[:])
                                else:
                                    nc.scalar.copy(osb[:], wo_ps[:])
                                nc.sync.dma_start(
                                    out=out[tcx * 128:(tcx + 1) * 128,
                                            dhalf * 1024:(dhalf + 1) * 1024],
                                    in_=osb[:])

    if split_waits:
        _split_multi_waits(nc, mybir)
    return nc


def _host_inputs(x, wq, wk, wv, wo):
    xT = np.ascontiguousarray(x.reshape(BT, D).T)
    half = DH // 2
    inv = (1.0 / (ROPE_BASE ** (np.arange(half, dtype=np.float32) / half))).astype(np.float32)
    ang = np.arange(T, dtype=np.float32)[:, None] * inv[None, :]          # (T, 64)
    c = np.cos(ang).T.astype(np.float32)                                  # (64, T)
    s = np.sin(ang).T.astype(np.float32)
    cosT = np.ascontiguousarray(np.concatenate([c, c], axis=0))           # (128, T)
    sinT = np.ascontiguousarray(np.concatenate([s, s], axis=0))
    rotMT = np.zeros((DH, DH), dtype=np.float32)
    rotMT[np.arange(64), np.arange(64) + 64] = 1.0    # lhsT: rotM[i+64, i] ... rot = rotM @ q
    rotMT[np.arange(64) + 64, np.arange(64)] = -1.0
    scale = np.float32(1.0 / np.sqrt(DH))
    in_maps = []
    for core in range(NCORES):
        kvh = core // 2
        in_maps.append({
            "xT": xT,
            "wqT": np.ascontiguousarray((wq[core * HPC * DH:(core + 1) * HPC * DH, :] * scale).T),
            "wkT": np.ascontiguousarray(wk[kvh * DH:(kvh + 1) * DH, :].T),
            "wvT": np.ascontiguousarray(wv[kvh * DH:(kvh + 1) * DH, :].T),
            "woT": np.ascontiguousarray(wo[:, core * HPC * DH:(core + 1) * HPC * DH].T),
            "cosT": cosT,
            "sinT": sinT,
            "rotMT": rotMT,
        })
    return in_maps


def kernel(x, wq, wk, wv, wo):
    _ensure_path()
    from concourse.bass_utils import run_bass_kernel_spmd

    x = np.asarray(x, dtype=np.float32)
    wq = np.asarray(wq, dtype=np.float32)
    wk = np.asarray(wk, dtype=np.float32)
    wv = np.asarray(wv, dtype=np.float32)
    wo = np.asarray(wo, dtype=np.float32)

    if "nc" not in _cache:
        _cache["nc"] = _build()
    nc = _cache["nc"]

    in_maps = _host_inputs(x, wq, wk, wv, wo)
    res = run_bass_kernel_spmd(nc, in_maps, list(range(NCORES)))
    acc = res.results[0]["out"].astype(np.float32)
    for cidx in range(1, NCORES):
        acc = acc + res.results[cidx]["out"]
    return acc.reshape(B, T, D)


# revision 38
# speedup vs baseline: 60594.8538x; 1.0683x over previous
"""Tensor-parallel GQA multi-head attention for 8 Trainium2 NeuronCores.

Sharding: query heads (16) split 2-per-core; each core needs exactly one
KV head (GQA group); wq/wk/wv column-parallel, wo row-parallel; the
all-reduce after wo is done host-side (sum of 8 partial outputs).

Per-core layout strategy: activations kept transposed (feature dim on
partitions, tokens on the free axis) so every matmul contracts over the
partition dim with N=512 streams:
  QT/KT = W^T-chunks (lhsT) x xT (rhs)         [dh, tokens]
  S^T   = KT-chunk (lhsT) x QT (rhs)           [s, t]  (causal superblocks)
  P^T   = exp(S^T + causal mask)               (no max-subtraction: scores
                                                are bounded ~N(0, 1/9))
  l     = ones x P^T (column sums via PE)      [1, t]
  avT   = V-chunk (lhsT) x P^T (rhs)           [dh, t]; scaled by 1/l
  out   = avT-chunk (lhsT) x woT (rhs)         [t, d] partial, DMA'd out
"""

import numpy as np

B, T, D, H, KV = 2, 2048, 2048, 16, 4
DH = 128
NCORES = 8
HPC = H // NCORES          # 2 query heads per core
BT = B * T                 # 4096
ND = D // 128              # 16 contraction chunks
NSB = T // 512             # 4 causal superblocks per batch
NTC = BT // 128            # 32 output token chunks
ROPE_BASE = 10000.0
NEG = -1.0e4

_cache = {}


def _ensure_path():
    try:
        import concourse.bass  # noqa: F401
    except ImportError:
        import sys
        for p in ("/opt/trn_rl_repo", "/root/.axon_site/_ro/trn_rl_repo"):
            if p not in sys.path:
                sys.path.insert(0, p)
        import concourse.bass  # noqa: F401


def _split_multi_waits(nc, mybir, max_waits=1):
    """This container's walrus rejects >1 sync-wait on one instruction
    (seen on the Tile tail drain). Move extra waits onto preceding NoOps
    on the same engine; per-engine program order preserves semantics."""
    for bb in nc.main_func.blocks:
        new_insts = []
        for ins in bb.instructions:
            si = getattr(ins, "sync_info", None)
            if si is not None and si.on_wait and len(si.on_wait) > max_waits:
                waits = list(si.on_wait)
                extra, keep = waits[:-max_waits], waits[-max_waits:]
                for w in extra:
                    new_insts.append(
                        mybir.InstNoOp(
                            name=nc.get_next_instruction_name(),
                            sync_info=mybir.SyncInfo(on_wait=[w], on_update=[]),
                            bass_nofuse=True,
                            engine=ins.engine,
                            ins=[],
                            outs=[],
                        )
                    )
                si.on_wait = keep
            new_insts.append(ins)
        bb.instructions = new_insts


def _build(split_waits=True, use_f32r=True):
    _ensure_path()
    import concourse.bass as bass
    import concourse.mybir as mybir
    import concourse.tile as tile
    from concourse.masks import make_identity

    f32 = mybir.dt.float32
    fr = mybir.dt.float32r if use_f32r else f32
    nc = bass.Bass()

    xT = nc.declare_dram_parameter("xT", [D, BT], fr, isOutput=False)
    wqT = nc.declare_dram_parameter("wqT", [D, HPC * DH], fr, isOutput=False)
    wkT = nc.declare_dram_parameter("wkT", [D, DH], fr, isOutput=False)
    wvT = nc.declare_dram_parameter("wvT", [D, DH], fr, isOutput=False)
    woT = nc.declare_dram_parameter("woT", [HPC * DH, D], fr, isOutput=False)
    cosT = nc.declare_dram_parameter("cosT", [DH, T], f32, isOutput=False)
    rotMT = nc.declare_dram_parameter("rotMT", [DH, DH], fr, isOutput=False)
    sinT = nc.declare_dram_parameter("sinT", [DH, T], f32, isOutput=False)
    out = nc.declare_dram_parameter("out", [BT, D], f32, isOutput=True)

    with nc.allow_low_precision(reason="float32r fast matmul path"), \
         tile.TileContext(nc) as tc:
        with tc.tile_pool(name="persist", bufs=1) as P:
            ident = P.tile([128, 128], f32, tag="ident")
            maskT = P.tile([128, 128], f32, tag="maskT")
            ones = P.tile([128, 1], fr, tag="ones")
            ones_r = P.tile([1, 128], fr, tag="ones_r")
            ones_f = P.tile([128, 1], f32, tag="ones_f")
            ones_rf = P.tile([1, 128], f32, tag="ones_rf")
            make_identity(nc, ident[:])
            # S^T diag block mask: keep (s_local - t_local) <= 0, else -1e4
            nc.gpsimd.memset(maskT[:], 0.0)
            # keep where (t_local - s_local) >= 0, i.e. s <= t
            nc.gpsimd.affine_select(
                out=maskT[:],
                in_=maskT[:],
                compare_op=mybir.AluOpType.is_ge,
                fill=NEG,
                base=0,
                pattern=[[1, 128]],
                channel_multiplier=-1,
            )
            nc.gpsimd.memset(ones_f[:], 1.0)
            nc.gpsimd.memset(ones_rf[:], 1.0)
            nc.vector.tensor_copy(ones[:], ones_f[:])
            nc.vector.tensor_copy(ones_r[:], ones_rf[:])

            rotm_sb = P.tile([128, 128], fr, tag="rotm")
            cos_sb = P.tile([128, T], f32, tag="cos")
            sin_sb = P.tile([128, T], f32, tag="sin")
            QT = [P.tile([128, BT], fr, tag=f"qt{h}", name=f"qt{h}") for h in range(HPC)]
            KT = P.tile([128, BT], fr, tag="kt")
            VT = P.tile([128, BT], f32, tag="vt")
            Vn = P.tile([128, BT], fr, tag="vn")
            AVT = [P.tile([128, BT], fr, tag=f"avt{h}", name=f"avt{h}") for h in range(HPC)]

            # ---------- phase A: QKV projections + RoPE ----------
            with tc.tile_pool(name="wpool", bufs=1) as WP, \
                 tc.tile_pool(name="xp", bufs=3) as XP, \
                 tc.tile_pool(name="ropetA", bufs=4) as RT2, \
                 tc.tile_pool(name="psA", bufs=1, space="PSUM") as PSA, \
                 tc.tile_pool(name="psScrA", bufs=4, space="PSUM") as PSCR:
                wq_sb = WP.tile([128, ND * HPC * DH], fr, tag="wq")
                wk_sb = WP.tile([128, ND * DH], fr, tag="wk")
                wv_sb = WP.tile([128, ND * DH], fr, tag="wv")
                # split weight loads so the first d-chunks land quickly
                for lo, hi in ((0, ND // 4), (ND // 4, ND)):
                    nc.sync.dma_start(
                        out=wq_sb[:, lo * 256: hi * 256].rearrange(
                            "p (c m) -> p c m", c=hi - lo),
                        in_=wqT[lo * 128: hi * 128, :].rearrange(
                            "(c p) m -> p c m", p=128))
                    nc.sync.dma_start(
                        out=wk_sb[:, lo * 128: hi * 128].rearrange(
                            "p (c m) -> p c m", c=hi - lo),
                        in_=wkT[lo * 128: hi * 128, :].rearrange(
                            "(c p) m -> p c m", p=128))
                    nc.sync.dma_start(
                        out=wv_sb[:, lo * 128: hi * 128].rearrange(
                            "p (c m) -> p c m", c=hi - lo),
                        in_=wvT[lo * 128: hi * 128, :].rearrange(
                            "(c p) m -> p c m", p=128))

                for tq5 in range(BT // 512):
                    pq = [PSA.tile([128, 512], f32, tag=f"pq{h}", name=f"pq{h}") for h in range(HPC)]
                    pk = PSA.tile([128, 512], f32, tag="pk")
                    pv = PSA.tile([128, 512], f32, tag="pv")
                    for dcg in range(4):
                        # one 1MB DMA: 4 d-chunks x 512 tokens
                        xt = XP.tile([128, 4 * 512], fr, tag="x")
                        nc.sync.dma_start(
                            out=xt[:].rearrange("p (c m) -> p c m", c=4),
                            in_=xT[dcg * 512:(dcg + 1) * 512,
                                   tq5 * 512:(tq5 + 1) * 512].rearrange(
                                       "(c p) m -> p c m", p=128))
                        if tq5 == 0 and dcg == 1:
                            # tables land after the first x tile so the first
                            # matmuls aren't queued behind them; ready well
                            # before the first inline rope needs them
                            nc.sync.dma_start(out=rotm_sb[:], in_=rotMT[:, :])
                            nc.sync.dma_start(out=cos_sb[:], in_=cosT[:, :])
                            nc.sync.dma_start(out=sin_sb[:], in_=sinT[:, :])
                        for dci in range(4):
                            dc = dcg * 4 + dci
                            xs = xt[:, dci * 512:(dci + 1) * 512]
                            st, sp = (dc == 0), (dc == ND - 1)
                            for h in range(HPC):
                                nc.tensor.matmul(
                                    pq[h][:],
                                    lhsT=wq_sb[:, dc * 256 + h * 128: dc * 256 + (h + 1) * 128],
                                    rhs=xs, start=st, stop=sp)
                            nc.tensor.matmul(
                                pk[:], lhsT=wk_sb[:, dc * 128:(dc + 1) * 128],
                                rhs=xs, start=st, stop=sp)
                            nc.tensor.matmul(
                                pv[:], lhsT=wv_sb[:, dc * 128:(dc + 1) * 128],
                                rhs=xs, start=st, stop=sp)
                    tcol = slice(tq5 * 512, (tq5 + 1) * 512)
                    for h in range(HPC):
                        nc.vector.tensor_copy(QT[h][:, tcol], pq[h][:, :])
                    nc.vector.tensor_copy(KT[:, tcol], pk[:, :])
                    nc.vector.tensor_copy(VT[:, tcol], pv[:, :])
                    # RoPE for this 512-token block, inline with projections
                    tab = slice((tq5 * 512) % T, (tq5 * 512) % T + 512)
                    for tgt in [KT, QT[0], QT[1]]:
                        rot_ps = PSCR.tile([128, 512], f32, tag="scr", name="rot_ps")
                        nc.tensor.matmul(rot_ps[:], lhsT=rotm_sb[:],
                                         rhs=tgt[:, tcol], start=True, stop=True)
                        rtmp = RT2.tile([128, 512], f32, tag="rtmp")
                        nc.vector.tensor_mul(rtmp[:], rot_ps[:], sin_sb[:, tab])
                        nc.vector.tensor_mul(tgt[:, tcol], tgt[:, tcol], cos_sb[:, tab])
                        nc.gpsimd.tensor_add(tgt[:, tcol], tgt[:, tcol], rtmp[:])
                    # V^T -> V natural for this block
                    for vc in range(tq5 * 4, (tq5 + 1) * 4):
                        vps = PSCR.tile([128, 128], f32, tag="scr", name="vtp")
                        nc.tensor.transpose(vps[:], VT[:, vc * 128:(vc + 1) * 128],
                                            ident[:])
                        nc.vector.tensor_copy(Vn[:, vc * 128:(vc + 1) * 128], vps[:])
            # ---------- phases A2/B0/B/C merged: rope, V-transpose,
            # attention, and interleaved output projection in one scope ------
            with tc.tile_pool(name="wop", bufs=1) as WOP, \
                 tc.tile_pool(name="ptp", bufs=4) as PTP, \
                 tc.tile_pool(name="rrp", bufs=2) as RRP, \
                 tc.tile_pool(name="osbp", bufs=3) as OSBP, \
                 tc.tile_pool(name="psScr", bufs=1, space="PSUM") as PSCR, \
                 tc.tile_pool(name="psSt", bufs=3, space="PSUM") as PSST, \
                 tc.tile_pool(name="psL", bufs=1, space="PSUM") as PSL, \
                 tc.tile_pool(name="psAv", bufs=1, space="PSUM") as PSAV, \
                 tc.tile_pool(name="psC", bufs=1, space="PSUM") as PSC:
                wo_sb = WOP.tile([128, HPC * D], fr, tag="wo")
                nc.sync.dma_start(
                    out=wo_sb[:].rearrange("p (c n) -> p c n", c=HPC),
                    in_=woT[:, :].rearrange("(c p) n -> p c n", p=128))
                # attention + output projection
                for b in range(B):
                    for tsb in range(NSB):
                        n_sc = (tsb + 1) * 4
                        tg = slice(b * T + tsb * 512, b * T + (tsb + 1) * 512)
                        for h in range(HPC):
                            av_ps = PSAV.tile([128, 512], f32, tag="av")
                            l_ps = PSL.tile([1, 512], f32, tag="l")
                            for sc in range(n_sc):
                                sc_rel = sc - tsb * 4
                                c0 = max(sc_rel, 0) * 128  # first valid t col
                                nv = slice(c0, 512)
                                tgn = slice(b * T + tsb * 512 + c0,
                                            b * T + (tsb + 1) * 512)
                                st_ps = PSST.tile([128, 512], f32, tag="st")
                                nc.tensor.matmul(
                                    st_ps[:, nv],
                                    lhsT=KT[:, b * T + sc * 128: b * T + (sc + 1) * 128],
                                    rhs=QT[h][:, tgn], start=True, stop=True)
                                if sc_rel >= 0:
                                    blk = st_ps[:, c0:c0 + 128]
                                    nc.vector.tensor_add(blk, blk, maskT[:])
                                pt = PTP.tile([128, 512], fr, tag="pt")
                                nc.scalar.activation(
                                    pt[:, nv], st_ps[:, nv],
                                    mybir.ActivationFunctionType.Exp)
                                nc.tensor.matmul(
                                    l_ps[:, nv], lhsT=ones[:], rhs=pt[:, nv],
                                    start=(sc == 0), stop=(sc == n_sc - 1))
                                nc.tensor.matmul(
                                    av_ps[:, nv],
                                    lhsT=Vn[:, b * T + sc * 128: b * T + (sc + 1) * 128],
                                    rhs=pt[:, nv], start=(sc == 0), stop=(sc == n_sc - 1))
                            rr = RRP.tile([1, 512], fr, tag="rr")
                            nc.vector.reciprocal(rr[:], l_ps[:])
                            nc.vector.tensor_copy(AVT[h][:, tg], av_ps[:])
                            rbc = PSCR.tile([128, 512], f32, tag="scr", name="rbc")
                            nc.tensor.matmul(
                                rbc[:], lhsT=ones_r[:], rhs=rr[:],
                                start=True, stop=True)
                            nc.vector.tensor_mul(AVT[h][:, tg], AVT[h][:, tg], rbc[:])
                        # both heads done for this 512-token group: project out
                        for tj in range(4):
                            tcx = (b * T + tsb * 512) // 128 + tj
                            for dhalf in range(2):
                                wo_ps = PSC.tile([128, D // 2], f32, tag="wops")
                                for h in range(HPC):
                                    for ndc in range(2):
                                        ns = slice(ndc * 512, (ndc + 1) * 512)
                                        nc.tensor.matmul(
                                            wo_ps[:, ns],
                                            lhsT=AVT[h][:, tcx * 128:(tcx + 1) * 128],
                                            rhs=wo_sb[:, h * D + dhalf * 1024 + ndc * 512:
                                                      h * D + dhalf * 1024 + (ndc + 1) * 512],
                                            start=(h == 0), stop=(h == HPC - 1))
                                osb = OSBP.tile([128, D // 2], f32, tag="osb")
                                if (tj + dhalf) % 2 == 0:
                                    nc.vector.tensor_copy(osb[:], wo_ps[:])
                                else:
                                    nc.scalar.copy(osb[:], wo_ps[:])
                                nc.sync.dma_start(
                                    out=out[tcx * 128:(tcx + 1) * 128,
                                            dhalf * 1024:(dhalf + 1) * 1024],
                                    in_=osb[:])

    if split_waits:
        _split_multi_waits(nc, mybir)
    return nc


def _host_inputs(x, wq, wk, wv, wo):
    xT = np.ascontiguousarray(x.reshape(BT, D).T)
    half = DH // 2
    inv = (1.0 / (ROPE_BASE ** (np.arange(half, dtype=np.float32) / half))).astype(np.float32)
    ang = np.arange(T, dtype=np.float32)[:, None] * inv[None, :]          # (T, 64)
    c = np.cos(ang).T.astype(np.float32)                                  # (64, T)
    s = np.sin(ang).T.astype(np.float32)
    cosT = np.ascontiguousarray(np.concatenate([c, c], axis=0))           # (128, T)
    sinT = np.ascontiguousarray(np.concatenate([s, s], axis=0))
    rotMT = np.zeros((DH, DH), dtype=np.float32)
    rotMT[np.arange(64), np.arange(64) + 64] = 1.0    # lhsT: rotM[i+64, i] ... rot = rotM @ q
    rotMT[np.arange(64) + 64, np.arange(64)] = -1.0
    scale = np.float32(1.0 / np.sqrt(DH))
    in_maps = []
    for core in range(NCORES):
        kvh = core // 2
        in_maps.append({
            "xT": xT,
            "wqT": np.ascontiguousarray((wq[core * HPC * DH:(core + 1) * HPC * DH, :] * scale).T),
            "wkT": np.ascontiguousarray(wk[kvh * DH:(kvh + 1) * DH, :].T),
            "wvT": np.ascontiguousarray(wv[kvh * DH:(kvh + 1) * DH, :].T),
            "woT": np.ascontiguousarray(wo[:, core * HPC * DH:(core + 1) * HPC * DH].T),
            "cosT": cosT,
            "sinT": sinT,
            "rotMT": rotMT,
        })
    return in_maps


def kernel(x, wq, wk, wv, wo):
    _ensure_path()
    from concourse.bass_utils import run_bass_kernel_spmd

    x = np.asarray(x, dtype=np.float32)
    wq = np.asarray(wq, dtype=np.float32)
    wk = np.asarray(wk, dtype=np.float32)
    wv = np.asarray(wv, dtype=np.float32)
    wo = np.asarray(wo, dtype=np.float32)

    if "nc" not in _cache:
        _cache["nc"] = _build()
    nc = _cache["nc"]

    in_maps = _host_inputs(x, wq, wk, wv, wo)
    res = run_bass_kernel_spmd(nc, in_maps, list(range(NCORES)))
    acc = res.results[0]["out"].astype(np.float32)
    for cidx in range(1, NCORES):
        acc = acc + res.results[cidx]["out"]
    return acc.reshape(B, T, D)


# revision 39
# speedup vs baseline: 60727.2430x; 1.0022x over previous
"""Tensor-parallel GQA multi-head attention for 8 Trainium2 NeuronCores.

Sharding: query heads (16) split 2-per-core; each core needs exactly one
KV head (GQA group); wq/wk/wv column-parallel, wo row-parallel; the
all-reduce after wo is done host-side (sum of 8 partial outputs).

Per-core layout strategy: activations kept transposed (feature dim on
partitions, tokens on the free axis) so every matmul contracts over the
partition dim with N=512 streams:
  QT/KT = W^T-chunks (lhsT) x xT (rhs)         [dh, tokens]
  S^T   = KT-chunk (lhsT) x QT (rhs)           [s, t]  (causal superblocks)
  P^T   = exp(S^T + causal mask)               (no max-subtraction: scores
                                                are bounded ~N(0, 1/9))
  l     = ones x P^T (column sums via PE)      [1, t]
  avT   = V-chunk (lhsT) x P^T (rhs)           [dh, t]; scaled by 1/l
  out   = avT-chunk (lhsT) x woT (rhs)         [t, d] partial, DMA'd out
"""

import numpy as np

B, T, D, H, KV = 2, 2048, 2048, 16, 4
DH = 128
NCORES = 8
HPC = H // NCORES          # 2 query heads per core
BT = B * T                 # 4096
ND = D // 128              # 16 contraction chunks
NSB = T // 512             # 4 causal superblocks per batch
NTC = BT // 128            # 32 output token chunks
ROPE_BASE = 10000.0
NEG = -1.0e4

_cache = {}


def _ensure_path():
    try:
        import concourse.bass  # noqa: F401
    except ImportError:
        import sys
        for p in ("/opt/trn_rl_repo", "/root/.axon_site/_ro/trn_rl_repo"):
            if p not in sys.path:
                sys.path.insert(0, p)
        import concourse.bass  # noqa: F401


def _split_multi_waits(nc, mybir, max_waits=1):
    """This container's walrus rejects >1 sync-wait on one instruction
    (seen on the Tile tail drain). Move extra waits onto preceding NoOps
    on the same engine; per-engine program order preserves semantics."""
    for bb in nc.main_func.blocks:
        new_insts = []
        for ins in bb.instructions:
            si = getattr(ins, "sync_info", None)
            if si is not None and si.on_wait and len(si.on_wait) > max_waits:
                waits = list(si.on_wait)
                extra, keep = waits[:-max_waits], waits[-max_waits:]
                for w in extra:
                    new_insts.append(
                        mybir.InstNoOp(
                            name=nc.get_next_instruction_name(),
                            sync_info=mybir.SyncInfo(on_wait=[w], on_update=[]),
                            bass_nofuse=True,
                            engine=ins.engine,
                            ins=[],
                            outs=[],
                        )
                    )
                si.on_wait = keep
            new_insts.append(ins)
        bb.instructions = new_insts


def _build(split_waits=True, use_f32r=True):
    _ensure_path()
    import concourse.bass as bass
    import concourse.mybir as mybir
    import concourse.tile as tile
    from concourse.masks import make_identity

    f32 = mybir.dt.float32
    fr = mybir.dt.float32r if use_f32r else f32
    nc = bass.Bass()

    xT = nc.declare_dram_parameter("xT", [D, BT], fr, isOutput=False)
    wqT = nc.declare_dram_parameter("wqT", [D, HPC * DH], fr, isOutput=False)
    wkT = nc.declare_dram_parameter("wkT", [D, DH], fr, isOutput=False)
    wvT = nc.declare_dram_parameter("wvT", [D, DH], fr, isOutput=False)
    woT = nc.declare_dram_parameter("woT", [HPC * DH, D], fr, isOutput=False)
    cosT = nc.declare_dram_parameter("cosT", [DH, T], f32, isOutput=False)
    rotMT = nc.declare_dram_parameter("rotMT", [DH, DH], fr, isOutput=False)
    sinT = nc.declare_dram_parameter("sinT", [DH, T], f32, isOutput=False)
    out = nc.declare_dram_parameter("out", [BT, D], f32, isOutput=True)

    with nc.allow_low_precision(reason="float32r fast matmul path"), \
         tile.TileContext(nc) as tc:
        with tc.tile_pool(name="persist", bufs=1) as P:
            ident = P.tile([128, 128], f32, tag="ident")
            maskT = P.tile([128, 128], f32, tag="maskT")
            ones = P.tile([128, 1], fr, tag="ones")
            ones_r = P.tile([1, 128], fr, tag="ones_r")
            ones_f = P.tile([128, 1], f32, tag="ones_f")
            ones_rf = P.tile([1, 128], f32, tag="ones_rf")
            make_identity(nc, ident[:])
            # S^T diag block mask: keep (s_local - t_local) <= 0, else -1e4
            nc.gpsimd.memset(maskT[:], 0.0)
            # keep where (t_local - s_local) >= 0, i.e. s <= t
            nc.gpsimd.affine_select(
                out=maskT[:],
                in_=maskT[:],
                compare_op=mybir.AluOpType.is_ge,
                fill=NEG,
                base=0,
                pattern=[[1, 128]],
                channel_multiplier=-1,
            )
            nc.gpsimd.memset(ones_f[:], 1.0)
            nc.gpsimd.memset(ones_rf[:], 1.0)
            nc.vector.tensor_copy(ones[:], ones_f[:])
            nc.vector.tensor_copy(ones_r[:], ones_rf[:])

            rotm_sb = P.tile([128, 128], fr, tag="rotm")
            cos_sb = P.tile([128, T], f32, tag="cos")
            sin_sb = P.tile([128, T], f32, tag="sin")
            QT = [P.tile([128, BT], fr, tag=f"qt{h}", name=f"qt{h}") for h in range(HPC)]
            KT = P.tile([128, BT], fr, tag="kt")
            VT = P.tile([128, BT], f32, tag="vt")
            Vn = P.tile([128, BT], fr, tag="vn")
            AVT = [P.tile([128, BT], fr, tag=f"avt{h}", name=f"avt{h}") for h in range(HPC)]

            # ---------- phase A: QKV projections + RoPE ----------
            with tc.tile_pool(name="wpool", bufs=1) as WP, \
                 tc.tile_pool(name="xp", bufs=3) as XP, \
                 tc.tile_pool(name="ropetA", bufs=4) as RT2, \
                 tc.tile_pool(name="psA", bufs=1, space="PSUM") as PSA, \
                 tc.tile_pool(name="psScrA", bufs=4, space="PSUM") as PSCR:
                wq_sb = WP.tile([128, ND * HPC * DH], fr, tag="wq")
                wk_sb = WP.tile([128, ND * DH], fr, tag="wk")
                wv_sb = WP.tile([128, ND * DH], fr, tag="wv")
                # split weight loads so the first d-chunks land quickly
                for lo, hi in ((0, ND // 4), (ND // 4, ND)):
                    nc.sync.dma_start(
                        out=wq_sb[:, lo * 256: hi * 256].rearrange(
                            "p (c m) -> p c m", c=hi - lo),
                        in_=wqT[lo * 128: hi * 128, :].rearrange(
                            "(c p) m -> p c m", p=128))
                    nc.sync.dma_start(
                        out=wk_sb[:, lo * 128: hi * 128].rearrange(
                            "p (c m) -> p c m", c=hi - lo),
                        in_=wkT[lo * 128: hi * 128, :].rearrange(
                            "(c p) m -> p c m", p=128))
                    nc.sync.dma_start(
                        out=wv_sb[:, lo * 128: hi * 128].rearrange(
                            "p (c m) -> p c m", c=hi - lo),
                        in_=wvT[lo * 128: hi * 128, :].rearrange(
                            "(c p) m -> p c m", p=128))

                for tq5 in range(BT // 512):
                    pq = [PSA.tile([128, 512], f32, tag=f"pq{h}", name=f"pq{h}") for h in range(HPC)]
                    pk = PSA.tile([128, 512], f32, tag="pk")
                    pv = PSA.tile([128, 512], f32, tag="pv")
                    for dcg in range(4):
                        # one 1MB DMA: 4 d-chunks x 512 tokens
                        xt = XP.tile([128, 4 * 512], fr, tag="x")
                        nc.sync.dma_start(
                            out=xt[:].rearrange("p (c m) -> p c m", c=4),
                            in_=xT[dcg * 512:(dcg + 1) * 512,
                                   tq5 * 512:(tq5 + 1) * 512].rearrange(
                                       "(c p) m -> p c m", p=128))
                        if tq5 == 0 and dcg == 1:
                            # tables land after the first x tile so the first
                            # matmuls aren't queued behind them; ready well
                            # before the first inline rope needs them
                            nc.sync.dma_start(out=rotm_sb[:], in_=rotMT[:, :])
                            nc.sync.dma_start(out=cos_sb[:], in_=cosT[:, :])
                            nc.sync.dma_start(out=sin_sb[:], in_=sinT[:, :])
                        for dci in range(4):
                            dc = dcg * 4 + dci
                            xs = xt[:, dci * 512:(dci + 1) * 512]
                            st, sp = (dc == 0), (dc == ND - 1)
                            for h in range(HPC):
                                nc.tensor.matmul(
                                    pq[h][:],
                                    lhsT=wq_sb[:, dc * 256 + h * 128: dc * 256 + (h + 1) * 128],
                                    rhs=xs, start=st, stop=sp)
                            nc.tensor.matmul(
                                pk[:], lhsT=wk_sb[:, dc * 128:(dc + 1) * 128],
                                rhs=xs, start=st, stop=sp)
                            nc.tensor.matmul(
                                pv[:], lhsT=wv_sb[:, dc * 128:(dc + 1) * 128],
                                rhs=xs, start=st, stop=sp)
                    tcol = slice(tq5 * 512, (tq5 + 1) * 512)
                    for h in range(HPC):
                        nc.vector.tensor_copy(QT[h][:, tcol], pq[h][:, :])
                    nc.vector.tensor_copy(KT[:, tcol], pk[:, :])
                    nc.vector.tensor_copy(VT[:, tcol], pv[:, :])
                    # RoPE for this 512-token block, inline with projections
                    tab = slice((tq5 * 512) % T, (tq5 * 512) % T + 512)
                    for tgt in [KT, QT[0], QT[1]]:
                        rot_ps = PSCR.tile([128, 512], f32, tag="scr", name="rot_ps")
                        nc.tensor.matmul(rot_ps[:], lhsT=rotm_sb[:],
                                         rhs=tgt[:, tcol], start=True, stop=True)
                        rtmp = RT2.tile([128, 512], f32, tag="rtmp")
                        nc.vector.tensor_mul(rtmp[:], rot_ps[:], sin_sb[:, tab])
                        nc.vector.tensor_mul(tgt[:, tcol], tgt[:, tcol], cos_sb[:, tab])
                        nc.gpsimd.tensor_add(tgt[:, tcol], tgt[:, tcol], rtmp[:])
                    # V^T -> V natural for this block
                    for vc in range(tq5 * 4, (tq5 + 1) * 4):
                        vps = PSCR.tile([128, 128], f32, tag="scr", name="vtp")
                        nc.tensor.transpose(vps[:], VT[:, vc * 128:(vc + 1) * 128],
                                            ident[:])
                        nc.vector.tensor_copy(Vn[:, vc * 128:(vc + 1) * 128], vps[:])
            # ---------- phases A2/B0/B/C merged: rope, V-transpose,
            # attention, and interleaved output projection in one scope ------
            with tc.tile_pool(name="wop", bufs=1) as WOP, \
                 tc.tile_pool(name="ptp", bufs=4) as PTP, \
                 tc.tile_pool(name="rrp", bufs=2) as RRP, \
                 tc.tile_pool(name="osbp", bufs=3) as OSBP, \
                 tc.tile_pool(name="psSt", bufs=3, space="PSUM") as PSST, \
                 tc.tile_pool(name="psL", bufs=1, space="PSUM") as PSL, \
                 tc.tile_pool(name="psAv", bufs=2, space="PSUM") as PSAV, \
                 tc.tile_pool(name="psC", bufs=1, space="PSUM") as PSC:
                wo_sb = WOP.tile([128, HPC * D], fr, tag="wo")
                nc.sync.dma_start(
                    out=wo_sb[:].rearrange("p (c n) -> p c n", c=HPC),
                    in_=woT[:, :].rearrange("(c p) n -> p c n", p=128))
                # attention + output projection
                for b in range(B):
                    for tsb in range(NSB):
                        n_sc = (tsb + 1) * 4
                        tg = slice(b * T + tsb * 512, b * T + (tsb + 1) * 512)
                        for h in range(HPC):
                            av_ps = PSAV.tile([128, 512], f32, tag="av")
                            l_ps = PSL.tile([1, 512], f32, tag="l")
                            for sc in range(n_sc):
                                sc_rel = sc - tsb * 4
                                c0 = max(sc_rel, 0) * 128  # first valid t col
                                nv = slice(c0, 512)
                                tgn = slice(b * T + tsb * 512 + c0,
                                            b * T + (tsb + 1) * 512)
                                st_ps = PSST.tile([128, 512], f32, tag="st")
                                nc.tensor.matmul(
                                    st_ps[:, nv],
                                    lhsT=KT[:, b * T + sc * 128: b * T + (sc + 1) * 128],
                                    rhs=QT[h][:, tgn], start=True, stop=True)
                                if sc_rel >= 0:
                                    blk = st_ps[:, c0:c0 + 128]
                                    nc.vector.tensor_add(blk, blk, maskT[:])
                                pt = PTP.tile([128, 512], fr, tag="pt")
                                nc.scalar.activation(
                                    pt[:, nv], st_ps[:, nv],
                                    mybir.ActivationFunctionType.Exp)
                                nc.tensor.matmul(
                                    l_ps[:, nv], lhsT=ones[:], rhs=pt[:, nv],
                                    start=(sc == 0), stop=(sc == n_sc - 1))
                                nc.tensor.matmul(
                                    av_ps[:, nv],
                                    lhsT=Vn[:, b * T + sc * 128: b * T + (sc + 1) * 128],
                                    rhs=pt[:, nv], start=(sc == 0), stop=(sc == n_sc - 1))
                            rr = RRP.tile([1, 512], fr, tag="rr")
                            nc.vector.reciprocal(rr[:], l_ps[:])
                            nc.vector.tensor_copy(AVT[h][:, tg], av_ps[:])
                            rbc = PSC.tile([128, 512], f32, tag="wops", name="rbc")
                            nc.tensor.matmul(
                                rbc[:], lhsT=ones_r[:], rhs=rr[:],
                                start=True, stop=True)
                            nc.vector.tensor_mul(AVT[h][:, tg], AVT[h][:, tg], rbc[:])
                        # both heads done for this 512-token group: project out
                        for tj in range(4):
                            tcx = (b * T + tsb * 512) // 128 + tj
                            for dhalf in range(2):
                                wo_ps = PSC.tile([128, D // 2], f32, tag="wops")
                                for h in range(HPC):
                                    for ndc in range(2):
                                        ns = slice(ndc * 512, (ndc + 1) * 512)
                                        nc.tensor.matmul(
                                            wo_ps[:, ns],
                                            lhsT=AVT[h][:, tcx * 128:(tcx + 1) * 128],
                                            rhs=wo_sb[:, h * D + dhalf * 1024 + ndc * 512:
                                                      h * D + dhalf * 1024 + (ndc + 1) * 512],
                                            start=(h == 0), stop=(h == HPC - 1))
                                osb = OSBP.tile([128, D // 2], f32, tag="osb")
                                if (tj + dhalf) % 2 == 0:
                                    nc.vector.tensor_copy(osb[:], wo_ps[:])
                                else:
                                    nc.scalar.copy(osb[:], wo_ps[:])
                                nc.sync.dma_start(
                                    out=out[tcx * 128:(tcx + 1) * 128,
                                            dhalf * 1024:(dhalf + 1) * 1024],
                                    in_=osb[:])

    if split_waits:
        _split_multi_waits(nc, mybir)
    return nc


def _host_inputs(x, wq, wk, wv, wo):
    xT = np.ascontiguousarray(x.reshape(BT, D).T)
    half = DH // 2
    inv = (1.0 / (ROPE_BASE ** (np.arange(half, dtype=np.float32) / half))).astype(np.float32)
    ang = np.arange(T, dtype=np.float32)[:, None] * inv[None, :]          # (T, 64)
    c = np.cos(ang).T.astype(np.float32)                                  # (64, T)
    s = np.sin(ang).T.astype(np.float32)
    cosT = np.ascontiguousarray(np.concatenate([c, c], axis=0))           # (128, T)
    sinT = np.ascontiguousarray(np.concatenate([s, s], axis=0))
    rotMT = np.zeros((DH, DH), dtype=np.float32)
    rotMT[np.arange(64), np.arange(64) + 64] = 1.0    # lhsT: rotM[i+64, i] ... rot = rotM @ q
    rotMT[np.arange(64) + 64, np.arange(64)] = -1.0
    scale = np.float32(1.0 / np.sqrt(DH))
    in_maps = []
    for core in range(NCORES):
        kvh = core // 2
        in_maps.append({
            "xT": xT,
            "wqT": np.ascontiguousarray((wq[core * HPC * DH:(core + 1) * HPC * DH, :] * scale).T),
            "wkT": np.ascontiguousarray(wk[kvh * DH:(kvh + 1) * DH, :].T),
            "wvT": np.ascontiguousarray(wv[kvh * DH:(kvh + 1) * DH, :].T),
            "woT": np.ascontiguousarray(wo[:, core * HPC * DH:(core + 1) * HPC * DH].T),
            "cosT": cosT,
            "sinT": sinT,
            "rotMT": rotMT,
        })
    return in_maps


def kernel(x, wq, wk, wv, wo):
    _ensure_path()
    from concourse.bass_utils import run_bass_kernel_spmd

    x = np.asarray(x, dtype=np.float32)
    wq = np.asarray(wq, dtype=np.float32)
    wk = np.asarray(wk, dtype=np.float32)
    wv = np.asarray(wv, dtype=np.float32)
    wo = np.asarray(wo, dtype=np.float32)

    if "nc" not in _cache:
        _cache["nc"] = _build()
    nc = _cache["nc"]

    in_maps = _host_inputs(x, wq, wk, wv, wo)
    res = run_bass_kernel_spmd(nc, in_maps, list(range(NCORES)))
    acc = res.results[0]["out"].astype(np.float32)
    for cidx in range(1, NCORES):
        acc = acc + res.results[cidx]["out"]
    return acc.reshape(B, T, D)


# revision 45
# speedup vs baseline: 61391.6716x; 1.0109x over previous
"""Tensor-parallel GQA multi-head attention for 8 Trainium2 NeuronCores.

Sharding: query heads (16) split 2-per-core; each core needs exactly one
KV head (GQA group); wq/wk/wv column-parallel, wo row-parallel; the
all-reduce after wo is done host-side (sum of 8 partial outputs).

Per-core layout strategy: activations kept transposed (feature dim on
partitions, tokens on the free axis) so every matmul contracts over the
partition dim with N=512 streams:
  QT/KT = W^T-chunks (lhsT) x xT (rhs)         [dh, tokens]
  S^T   = KT-chunk (lhsT) x QT (rhs)           [s, t]  (causal superblocks)
  P^T   = exp(S^T + causal mask)               (no max-subtraction: scores
                                                are bounded ~N(0, 1/9))
  l     = ones x P^T (column sums via PE)      [1, t]
  avT   = V-chunk (lhsT) x P^T (rhs)           [dh, t]; scaled by 1/l
  out   = avT-chunk (lhsT) x woT (rhs)         [t, d] partial, DMA'd out
"""

import numpy as np

B, T, D, H, KV = 2, 2048, 2048, 16, 4
DH = 128
NCORES = 8
HPC = H // NCORES          # 2 query heads per core
BT = B * T                 # 4096
ND = D // 128              # 16 contraction chunks
NSB = T // 512             # 4 causal superblocks per batch
NTC = BT // 128            # 32 output token chunks
ROPE_BASE = 10000.0
NEG = -1.0e4

_cache = {}


def _ensure_path():
    try:
        import concourse.bass  # noqa: F401
    except ImportError:
        import sys
        for p in ("/opt/trn_rl_repo", "/root/.axon_site/_ro/trn_rl_repo"):
            if p not in sys.path:
                sys.path.insert(0, p)
        import concourse.bass  # noqa: F401


def _split_multi_waits(nc, mybir, max_waits=1):
    """This container's walrus rejects >1 sync-wait on one instruction
    (seen on the Tile tail drain). Move extra waits onto preceding NoOps
    on the same engine; per-engine program order preserves semantics."""
    for bb in nc.main_func.blocks:
        new_insts = []
        for ins in bb.instructions:
            si = getattr(ins, "sync_info", None)
            if si is not None and si.on_wait and len(si.on_wait) > max_waits:
                waits = list(si.on_wait)
                extra, keep = waits[:-max_waits], waits[-max_waits:]
                for w in extra:
                    new_insts.append(
                        mybir.InstNoOp(
                            name=nc.get_next_instruction_name(),
                            sync_info=mybir.SyncInfo(on_wait=[w], on_update=[]),
                            bass_nofuse=True,
                            engine=ins.engine,
                            ins=[],
                            outs=[],
                        )
                    )
                si.on_wait = keep
            new_insts.append(ins)
        bb.instructions = new_insts


def _build(split_waits=True, use_f32r=True):
    _ensure_path()
    import concourse.bass as bass
    import concourse.mybir as mybir
    import concourse.tile as tile
    from concourse.masks import make_identity

    f32 = mybir.dt.float32
    fr = mybir.dt.float32r if use_f32r else f32
    nc = bass.Bass()

    xT = nc.declare_dram_parameter("xT", [D, BT], fr, isOutput=False)
    wqT = nc.declare_dram_parameter("wqT", [D, HPC * DH], fr, isOutput=False)
    wkT = nc.declare_dram_parameter("wkT", [D, DH], fr, isOutput=False)
    wvT = nc.declare_dram_parameter("wvT", [D, DH], fr, isOutput=False)
    woT = nc.declare_dram_parameter("woT", [HPC * DH, D], fr, isOutput=False)
    cosT = nc.declare_dram_parameter("cosT", [DH, T], f32, isOutput=False)
    rotMT = nc.declare_dram_parameter("rotMT", [DH, DH], fr, isOutput=False)
    sinT = nc.declare_dram_parameter("sinT", [DH, T], f32, isOutput=False)
    out = nc.declare_dram_parameter("out", [BT, D], f32, isOutput=True)

    with nc.allow_low_precision(reason="float32r fast matmul path"), \
         tile.TileContext(nc) as tc:
        with tc.tile_pool(name="persist", bufs=1) as P:
            ident = P.tile([128, 128], f32, tag="ident")
            maskT = P.tile([128, 128], f32, tag="maskT")
            ones = P.tile([128, 1], fr, tag="ones")
            ones_r = P.tile([1, 128], fr, tag="ones_r")
            ones_f = P.tile([128, 1], f32, tag="ones_f")
            ones_rf = P.tile([1, 128], f32, tag="ones_rf")
            make_identity(nc, ident[:])
            # S^T diag block mask: keep (s_local - t_local) <= 0, else -1e4
            nc.gpsimd.memset(maskT[:], 0.0)
            # keep where (t_local - s_local) >= 0, i.e. s <= t
            nc.gpsimd.affine_select(
                out=maskT[:],
                in_=maskT[:],
                compare_op=mybir.AluOpType.is_ge,
                fill=NEG,
                base=0,
                pattern=[[1, 128]],
                channel_multiplier=-1,
            )
            nc.gpsimd.memset(ones_f[:], 1.0)
            nc.gpsimd.memset(ones_rf[:], 1.0)
            nc.vector.tensor_copy(ones[:], ones_f[:])
            nc.vector.tensor_copy(ones_r[:], ones_rf[:])

            rotm_sb = P.tile([128, 128], fr, tag="rotm")
            cos_sb = P.tile([128, T], f32, tag="cos")
            sin_sb = P.tile([128, T], f32, tag="sin")
            QT = [P.tile([128, BT], fr, tag=f"qt{h}", name=f"qt{h}") for h in range(HPC)]
            KT = P.tile([128, BT], fr, tag="kt")
            VT = P.tile([128, BT], f32, tag="vt")
            Vn = P.tile([128, BT], fr, tag="vn")
            AVT = [P.tile([128, BT], fr, tag=f"avt{h}", name=f"avt{h}") for h in range(HPC)]

            # ---------- phase A: QKV projections + RoPE ----------
            with tc.tile_pool(name="wpool", bufs=1) as WP, \
                 tc.tile_pool(name="xp", bufs=3) as XP, \
                 tc.tile_pool(name="ropetA", bufs=4) as RT2, \
                 tc.tile_pool(name="psA", bufs=1, space="PSUM") as PSA, \
                 tc.tile_pool(name="psScrA", bufs=4, space="PSUM") as PSCR:
                wq_sb = WP.tile([128, ND * HPC * DH], fr, tag="wq")
                wk_sb = WP.tile([128, ND * DH], fr, tag="wk")
                wv_sb = WP.tile([128, ND * DH], fr, tag="wv")
                # split weight loads so the first d-chunks land quickly
                for lo, hi in ((0, ND // 4), (ND // 4, ND)):
                    nc.sync.dma_start(
                        out=wq_sb[:, lo * 256: hi * 256].rearrange(
                            "p (c m) -> p c m", c=hi - lo),
                        in_=wqT[lo * 128: hi * 128, :].rearrange(
                            "(c p) m -> p c m", p=128))
                    nc.sync.dma_start(
                        out=wk_sb[:, lo * 128: hi * 128].rearrange(
                            "p (c m) -> p c m", c=hi - lo),
                        in_=wkT[lo * 128: hi * 128, :].rearrange(
                            "(c p) m -> p c m", p=128))
                    nc.sync.dma_start(
                        out=wv_sb[:, lo * 128: hi * 128].rearrange(
                            "p (c m) -> p c m", c=hi - lo),
                        in_=wvT[lo * 128: hi * 128, :].rearrange(
                            "(c p) m -> p c m", p=128))

                for tq5 in range(BT // 512):
                    pq = [PSA.tile([128, 512], f32, tag=f"pq{h}", name=f"pq{h}") for h in range(HPC)]
                    pk = PSA.tile([128, 512], f32, tag="pk")
                    pv = PSA.tile([128, 512], f32, tag="pv")
                    for dcg in range(4):
                        # one 1MB DMA: 4 d-chunks x 512 tokens
                        xt = XP.tile([128, 4 * 512], fr, tag="x")
                        nc.sync.dma_start(
                            out=xt[:].rearrange("p (c m) -> p c m", c=4),
                            in_=xT[dcg * 512:(dcg + 1) * 512,
                                   tq5 * 512:(tq5 + 1) * 512].rearrange(
                                       "(c p) m -> p c m", p=128))
                        if tq5 == 0 and dcg == 1:
                            # tables land after the first x tile so the first
                            # matmuls aren't queued behind them; ready well
                            # before the first inline rope needs them
                            nc.sync.dma_start(out=rotm_sb[:], in_=rotMT[:, :])
                            nc.sync.dma_start(out=cos_sb[:], in_=cosT[:, :])
                            nc.sync.dma_start(out=sin_sb[:], in_=sinT[:, :])
                        for dci in range(4):
                            dc = dcg * 4 + dci
                            xs = xt[:, dci * 512:(dci + 1) * 512]
                            st, sp = (dc == 0), (dc == ND - 1)
                            for h in range(HPC):
                                nc.tensor.matmul(
                                    pq[h][:],
                                    lhsT=wq_sb[:, dc * 256 + h * 128: dc * 256 + (h + 1) * 128],
                                    rhs=xs, start=st, stop=sp)
                            nc.tensor.matmul(
                                pk[:], lhsT=wk_sb[:, dc * 128:(dc + 1) * 128],
                                rhs=xs, start=st, stop=sp)
                            nc.tensor.matmul(
                                pv[:], lhsT=wv_sb[:, dc * 128:(dc + 1) * 128],
                                rhs=xs, start=st, stop=sp)
                    tcol = slice(tq5 * 512, (tq5 + 1) * 512)
                    for h in range(HPC):
                        nc.vector.tensor_copy(QT[h][:, tcol], pq[h][:, :])
                    nc.vector.tensor_copy(KT[:, tcol], pk[:, :])
                    nc.vector.tensor_copy(VT[:, tcol], pv[:, :])
                    # RoPE for this 512-token block, inline with projections
                    tab = slice((tq5 * 512) % T, (tq5 * 512) % T + 512)
                    for tgt in [KT, QT[0], QT[1]]:
                        rot_ps = PSCR.tile([128, 512], f32, tag="scr", name="rot_ps")
                        nc.tensor.matmul(rot_ps[:], lhsT=rotm_sb[:],
                                         rhs=tgt[:, tcol], start=True, stop=True)
                        rtmp = RT2.tile([128, 512], f32, tag="rtmp")
                        nc.vector.tensor_mul(rtmp[:], rot_ps[:], sin_sb[:, tab])
                        nc.vector.tensor_mul(tgt[:, tcol], tgt[:, tcol], cos_sb[:, tab])
                        nc.gpsimd.tensor_add(tgt[:, tcol], tgt[:, tcol], rtmp[:])
                    # V^T -> V natural for this block
                    for vc in range(tq5 * 4, (tq5 + 1) * 4):
                        vps = PSCR.tile([128, 128], f32, tag="scr", name="vtp")
                        nc.tensor.transpose(vps[:], VT[:, vc * 128:(vc + 1) * 128],
                                            ident[:])
                        nc.vector.tensor_copy(Vn[:, vc * 128:(vc + 1) * 128], vps[:])
            # ---------- phases A2/B0/B/C merged: rope, V-transpose,
            # attention, and interleaved output projection in one scope ------
            with tc.tile_pool(name="wop", bufs=1) as WOP, \
                 tc.tile_pool(name="ptp", bufs=4) as PTP, \
                 tc.tile_pool(name="rrp", bufs=2) as RRP, \
                 tc.tile_pool(name="osbp", bufs=3) as OSBP, \
                 tc.tile_pool(name="psSt", bufs=4, space="PSUM") as PSST, \
                 tc.tile_pool(name="psL", bufs=1, space="PSUM") as PSL, \
                 tc.tile_pool(name="psAv", bufs=1, space="PSUM") as PSAV, \
                 tc.tile_pool(name="psC", bufs=1, space="PSUM") as PSC:
                wo_sb = WOP.tile([128, HPC * D], fr, tag="wo")
                nc.sync.dma_start(
                    out=wo_sb[:].rearrange("p (c n) -> p c n", c=HPC),
                    in_=woT[:, :].rearrange("(c p) n -> p c n", p=128))
                # attention + output projection
                # keep the longest group (tsb=3) off the tail position
                groups = [(0, 0), (0, 1), (0, 2), (0, 3),
                          (1, 0), (1, 3), (1, 1), (1, 2)]
                for b, tsb in groups:
                    if True:
                        n_sc = (tsb + 1) * 4
                        tg = slice(b * T + tsb * 512, b * T + (tsb + 1) * 512)
                        for h in range(HPC):
                            av_ps = PSAV.tile([128, 512], f32, tag="av")
                            l_ps = PSL.tile([1, 512], f32, tag="l")
                            for sc in range(n_sc):
                                sc_rel = sc - tsb * 4
                                c0 = max(sc_rel, 0) * 128  # first valid t col
                                nv = slice(c0, 512)
                                tgn = slice(b * T + tsb * 512 + c0,
                                            b * T + (tsb + 1) * 512)
                                st_ps = PSST.tile([128, 512], f32, tag="st")
                                nc.tensor.matmul(
                                    st_ps[:, nv],
                                    lhsT=KT[:, b * T + sc * 128: b * T + (sc + 1) * 128],
                                    rhs=QT[h][:, tgn], start=True, stop=True)
                                if sc_rel >= 0:
                                    blk = st_ps[:, c0:c0 + 128]
                                    nc.vector.tensor_add(blk, blk, maskT[:])
                                pt = PTP.tile([128, 512], fr, tag="pt")
                                nc.scalar.activation(
                                    pt[:, nv], st_ps[:, nv],
                                    mybir.ActivationFunctionType.Exp)
                                nc.tensor.matmul(
                                    l_ps[:, nv], lhsT=ones[:], rhs=pt[:, nv],
                                    start=(sc == 0), stop=(sc == n_sc - 1))
                                nc.tensor.matmul(
                                    av_ps[:, nv],
                                    lhsT=Vn[:, b * T + sc * 128: b * T + (sc + 1) * 128],
                                    rhs=pt[:, nv], start=(sc == 0), stop=(sc == n_sc - 1))
                            rr = RRP.tile([1, 512], fr, tag="rr")
                            nc.vector.reciprocal(rr[:], l_ps[:])
                            nc.vector.tensor_copy(AVT[h][:, tg], av_ps[:])
                            rbc = PSC.tile([128, 512], f32, tag="wops", name="rbc")
                            nc.tensor.matmul(
                                rbc[:], lhsT=ones_r[:], rhs=rr[:],
                                start=True, stop=True)
                            nc.vector.tensor_mul(AVT[h][:, tg], AVT[h][:, tg], rbc[:])
                        # both heads done for this 512-token group: project out
                        for tj in range(4):
                            tcx = (b * T + tsb * 512) // 128 + tj
                            for dhalf in range(2):
                                wo_ps = PSC.tile([128, D // 2], f32, tag="wops")
                                for h in range(HPC):
                                    for ndc in range(2):
                                        ns = slice(ndc * 512, (ndc + 1) * 512)
                                        nc.tensor.matmul(
                                            wo_ps[:, ns],
                                            lhsT=AVT[h][:, tcx * 128:(tcx + 1) * 128],
                                            rhs=wo_sb[:, h * D + dhalf * 1024 + ndc * 512:
                                                      h * D + dhalf * 1024 + (ndc + 1) * 512],
                                            start=(h == 0), stop=(h == HPC - 1))
                                osb = OSBP.tile([128, D // 2], f32, tag="osb")
                                if (tj + dhalf) % 2 == 0:
                                    nc.vector.tensor_copy(osb[:], wo_ps[:])
                                else:
                                    nc.scalar.copy(osb[:], wo_ps[:])
                                nc.sync.dma_start(
                                    out=out[tcx * 128:(tcx + 1) * 128,
                                            dhalf * 1024:(dhalf + 1) * 1024],
                                    in_=osb[:])

    if split_waits:
        _split_multi_waits(nc, mybir)
    return nc


def _host_inputs(x, wq, wk, wv, wo):
    xT = np.ascontiguousarray(x.reshape(BT, D).T)
    half = DH // 2
    inv = (1.0 / (ROPE_BASE ** (np.arange(half, dtype=np.float32) / half))).astype(np.float32)
    ang = np.arange(T, dtype=np.float32)[:, None] * inv[None, :]          # (T, 64)
    c = np.cos(ang).T.astype(np.float32)                                  # (64, T)
    s = np.sin(ang).T.astype(np.float32)
    cosT = np.ascontiguousarray(np.concatenate([c, c], axis=0))           # (128, T)
    sinT = np.ascontiguousarray(np.concatenate([s, s], axis=0))
    rotMT = np.zeros((DH, DH), dtype=np.float32)
    rotMT[np.arange(64), np.arange(64) + 64] = 1.0    # lhsT: rotM[i+64, i] ... rot = rotM @ q
    rotMT[np.arange(64) + 64, np.arange(64)] = -1.0
    scale = np.float32(1.0 / np.sqrt(DH))
    in_maps = []
    for core in range(NCORES):
        kvh = core // 2
        in_maps.append({
            "xT": xT,
            "wqT": np.ascontiguousarray((wq[core * HPC * DH:(core + 1) * HPC * DH, :] * scale).T),
            "wkT": np.ascontiguousarray(wk[kvh * DH:(kvh + 1) * DH, :].T),
            "wvT": np.ascontiguousarray(wv[kvh * DH:(kvh + 1) * DH, :].T),
            "woT": np.ascontiguousarray(wo[:, core * HPC * DH:(core + 1) * HPC * DH].T),
            "cosT": cosT,
            "sinT": sinT,
            "rotMT": rotMT,
        })
    return in_maps


def kernel(x, wq, wk, wv, wo):
    _ensure_path()
    from concourse.bass_utils import run_bass_kernel_spmd

    x = np.asarray(x, dtype=np.float32)
    wq = np.asarray(wq, dtype=np.float32)
    wk = np.asarray(wk, dtype=np.float32)
    wv = np.asarray(wv, dtype=np.float32)
    wo = np.asarray(wo, dtype=np.float32)

    if "nc" not in _cache:
        _cache["nc"] = _build()
    nc = _cache["nc"]

    in_maps = _host_inputs(x, wq, wk, wv, wo)
    res = run_bass_kernel_spmd(nc, in_maps, list(range(NCORES)))
    acc = res.results[0]["out"].astype(np.float32)
    for cidx in range(1, NCORES):
        acc = acc + res.results[cidx]["out"]
    return acc.reshape(B, T, D)
